# revision 28
# baseline (speedup 1.0000x reference)
"""BertSelfAttention (ALiBi-style additive bias) on 8 TRN2 NeuronCores.

Problem: B=4, S=1024, D=1024, H=16 heads (HD=64), fp32.
  qkv = hidden @ Wqkv_w.T + Wqkv_b
  scores = q @ k.T / sqrt(64) + bias ;  probs = softmax(scores) ; out = probs @ v

Sharding: 8 cores = 4 batches x 2 head-groups of 8 "slots". Core c handles
batch c//2 and takes one head from each of 8 head PAIRS (parity c%2), so both
cores of a batch run the identical program on equal work.

Key ideas vs the naive kernel:
  * exp(s + b) = exp(s) * exp(b): the additive bias never touches the
    TensorEngine. The host ships exp(bias) (bf16); the device multiplies it
    into exp(scores) on the (otherwise idle) DVE. This removes 128 identity
    matmuls (~35us of PE time) per core.
  * ALiBi block sparsity: bias = -slope_h * |q - k|, so (128x128) score
    blocks with bias < -T contribute < e^-T relative softmax mass and are
    skipped entirely (no QK matmul, no exp, no bias DMA, no AV matmul).
    Heads are paired sparse-with-sparse across the two cores of a batch so
    the shared SPMD program uses per-slot block radii = max over the pair.
    kernel() verifies the dead blocks against the ACTUAL runtime bias values
    and falls back to a dense variant of the same program if the input is
    not ALiBi-shaped.
  * Scores are computed transposed (scoresT[k, q]) so the AV matmul
    [v | 1].T @ expT also yields the softmax denominator in row 64;
    normalization = fast reciprocal + partition-broadcast + DVE multiply.
  * No max-subtraction in softmax: |scores| <= ~10 in fp32 cannot overflow,
    and large-negative ALiBi bias underflows exp to a clean 0.
  * DMA order: the 8 hidden/weight chunks are triggered first (constants
    after) so the first matmul starts as soon as chunk 0 lands.
"""

import numpy as np

import concourse.bacc as bacc
import concourse.bass as bass
import concourse.mybir as mybir
from concourse.tile import TileContext

B, S, D = 4, 1024, 1024
H = 16
HD = 64  # head dim
N_CORES = 8
HPC = 8  # head-slots per core
OC = 3 * HPC * HD  # 1536 fused-qkv output rows per core
F32 = mybir.dt.float32
BF16 = mybir.dt.bfloat16

KC = S // 128  # 8 key-token chunks of 128
DC = D // 128  # 8 contraction chunks of 128

# --- ALiBi sparsity geometry -------------------------------------------------
# Head h (0-indexed) has slope 2^(-8(h+1)/16). A (kc, qc) block of 128x128
# token pairs is dead when its *smallest* |q - k| distance, 128*|kc-qc| - 127,
# gives bias below -T_SPARSE everywhere in the block.
T_SPARSE = 6.0
T_CHECK = 5.5  # runtime verification margin for dead blocks
_DENSE_MIN_R = 4  # pair radii above 3 round up to fully dense (r=7)


def _alibi_radii(thresh: float) -> list[int]:
    slopes = 2.0 ** (-8.0 * (np.arange(1, H + 1) / H))
    radii = []
    for sl in slopes:
        r = 7
        while r >= 1 and 128 * r - 127 > thresh / sl:
            r -= 1
        radii.append(r)
    return radii


def _make_slots(radii: list[int]):
    order = sorted(range(H), key=lambda h: (radii[h], h))
    pairs = [(order[2 * i], order[2 * i + 1]) for i in range(HPC)]
    slot_r = [max(radii[a], radii[b]) for a, b in pairs]
    # densest slots first: the tail (last slot's serial normalize chain)
    # then belongs to the sparsest head, and early slots line up with the
    # first-finished qkv bands.
    perm = sorted(range(HPC), key=lambda i: (-slot_r[i], i))
    # PSUM accumulation groups must have a uniform footprint (region-varying
    # start/stop within a bank silently drops contributions), so sparse slots
    # use per-query-block accumulation groups. That only pays off for small
    # radii; near-dense slots round up to the plain dense pattern.
    slot_r = [r if r < _DENSE_MIN_R else KC - 1 for r in slot_r]
    return [pairs[i] for i in perm], [slot_r[i] for i in perm]


PAIRS, SLOT_R = _make_slots(_alibi_radii(T_SPARSE))
DENSE_R = [7] * HPC


def _window(r: int, kc: int) -> tuple[int, int]:
    """Alive query-block range [lo, hi] (inclusive) for key-chunk kc."""
    return max(0, kc - r), min(KC - 1, kc + r)


def _half_ranges(w0: int, w1: int):
    """Split column range [w0, w1) at the 512 PSUM-bank boundary."""
    out = []
    for hb in range(2):
        c0, c1 = max(w0, hb * 512), min(w1, (hb + 1) * 512)
        if c0 < c1:
            out.append((hb, c0, c1))
    return out


def build_bass(slot_r=None) -> bass.Bass:
    if slot_r is None:
        slot_r = SLOT_R
    nc = bacc.Bacc()

    hw = nc.declare_dram_parameter("hw", [D, S + OC], BF16, isOutput=False)
    wbv = nc.declare_dram_parameter("wbv", [128, HPC * HD], BF16, isOutput=False)
    wbp = nc.declare_dram_parameter("wbp", [128, 8], F32, isOutput=False)
    ebT = nc.declare_dram_parameter("ebT", [HPC, S, S], BF16, isOutput=False)
    oT = nc.declare_dram_parameter("oT", [HPC * HD, S], F32, isOutput=True)

    with TileContext(nc) as tc:
        with (
            tc.tile_pool(name="const", bufs=1) as constp,
            tc.tile_pool(name="weights", bufs=1) as wp,
            tc.tile_pool(name="qk", bufs=1) as qkp,
            tc.tile_pool(name="vex", bufs=1) as vp,
            tc.tile_pool(name="ebias", bufs=8) as ebp,
            tc.tile_pool(name="exp", bufs=5) as ep,
            # sparse qc-major AV reads et2(kc) until qc = kc + 2r + LAG,
            # so keep enough rotation depth for r=3 slots
            tc.tile_pool(name="exp2", bufs=12) as e2p,
            tc.tile_pool(name="outs", bufs=2) as op_,
            tc.tile_pool(name="ps_mm", bufs=2, space="PSUM") as ps_mm,
            tc.tile_pool(name="ps_sm", bufs=4, space="PSUM") as ps_sm,
        ):
            # --- stage inputs ---------------------------------------------
            # hidden^T | W^T chunks first: the first matmul only waits on
            # chunk 0. Small constants ride behind them on the queue.
            hT_sb = []
            wT_sb = []
            for c in range(DC):
                hwt = wp.tile([128, S + OC], BF16, tag=f"hw{c}", name=f"hw{c}")
                nc.sync.dma_start(out=hwt[:], in_=hw[c * 128 : (c + 1) * 128, :])
                hT_sb.append(hwt[:, 0:S])
                wT_sb.append(hwt[:, S : S + OC])

            wbv_sb = constp.tile([128, HPC, HD], BF16)
            nc.sync.dma_start(
                out=wbv_sb[:].rearrange("p h d -> p (h d)"), in_=wbv[:]
            )
            wbp_sb = constp.tile([128, 8], F32)
            nc.sync.dma_start(out=wbp_sb[:], in_=wbp[:])

            # --- phase 1: fused QKV projection -----------------------------
            # qk_sb[j][p, t]: j in 0..3 -> q rows (pre-scaled by 1/8),
            #                 j in 4..7 -> k rows. Row (j%4)*128+p = oc index.
            qk_sb = [
                qkp.tile([128, S], BF16, tag=f"qk{j}", name=f"qk{j}")
                for j in range(8)
            ]
            # v_sb[t][p, s, 0:64] = v slot s, token t*128+p; [.., 64] = 1.0
            v_sb = [
                vp.tile([128, HPC, HD + 1], BF16, tag=f"vx{t}", name=f"v{t}")
                for t in range(KC)
            ]

            def qk_blk(j):
                ps = ps_mm.tile([128, S], F32, tag="mm", name=f"qkp{j}")

                def mm(c):
                    lw = wT_sb[c][:, j * 128 : (j + 1) * 128]
                    for half in range(2):
                        nc.tensor.matmul(
                            ps[:, half * 512 : (half + 1) * 512],
                            lw,
                            hT_sb[c][:, half * 512 : (half + 1) * 512],
                            start=(c == 0),
                            stop=(c == DC - 1),
                        )

                def fin():
                    # copy to SBUF, adding the per-partition qkv bias and
                    # folding the 1/sqrt(HD) score scale into q rows (DVE)
                    if j < 4:
                        nc.vector.tensor_scalar(
                            qk_sb[j][:], ps[:], wbp_sb[:, j : j + 1], 0.125,
                            op0=mybir.AluOpType.add, op1=mybir.AluOpType.mult,
                        )
                    else:
                        nc.vector.tensor_scalar_add(
                            qk_sb[j][:], ps[:], wbp_sb[:, j : j + 1]
                        )

                return mm, fin

            def v_blk(t):
                ps = ps_sm.tile([128, HPC * HD], F32, tag="sm", name=f"vps{t}")

                def mm(c):
                    nc.tensor.matmul(
                        ps[:],
                        hT_sb[c][:, t * 128 : (t + 1) * 128],
                        wT_sb[c][:, 2 * HPC * HD : 3 * HPC * HD],
                        start=(c == 0),
                        stop=(c == DC - 1),
                    )

                def fin():
                    nc.vector.tensor_tensor(
                        v_sb[t][:, :, 0:HD],
                        ps[:].rearrange("p (h d) -> p h d", h=HPC),
                        wbv_sb[:],
                        op=mybir.AluOpType.add,
                    )
                    nc.scalar.activation(
                        v_sb[t][:, :, HD : HD + 1],
                        v_sb[t][:, :, 0:1],
                        mybir.ActivationFunctionType.Identity,
                        scale=0.0,
                        bias=1.0,
                    )

                return mm, fin

            bands = [
                [qk_blk(0), qk_blk(4), v_blk(0), v_blk(1), v_blk(2)],
                [qk_blk(1), qk_blk(5), v_blk(3), v_blk(4), v_blk(5)],
                [qk_blk(2), qk_blk(6), v_blk(6), v_blk(7)],
                [qk_blk(3), qk_blk(7)],
            ]
            for band in bands:
                for c in range(DC):
                    for mm, _ in band:
                        mm(c)
                for _, fin in band:
                    fin()

            # --- phase 2: attention ----------------------------------------
            # Software-pipelined across (slot, k-chunk) items: the AV matmuls
            # are emitted LAG items late so the in-order PE stream never
            # stalls waiting on an item's exp * exp(bias).
            #
            # Dense slots (r=7) accumulate kc-major with full-half groups.
            # Sparse slots accumulate qc-major: per query-block, one
            # uniform-footprint PSUM group over its alive key-chunks
            # (region-varying start/stop within a group loses contributions).
            # Items are slot-sequential except the last two (sparsest) slots,
            # which interleave so their end-of-kernel drains overlap.
            items = [(s, kc) for s in range(HPC - 2) for kc in range(KC)]
            for kc in range(KC):
                items.append((HPC - 2, kc))
                items.append((HPC - 1, kc))
            idx_of = {it: i for i, it in enumerate(items)}
            et2s: dict[tuple, object] = {}
            pos_map: dict[int, list] = {}

            def emit_front(i):
                s, kc = items[i]
                r = slot_r[s]
                lo, hi = _window(r, kc)
                w0, w1 = lo * 128, (hi + 1) * 128
                j, po = s // 2, (s % 2) * 64
                qT = qk_sb[j][po : po + 64, :]  # [64, S] (already /8)
                kT = qk_sb[4 + j][po : po + 64, :]  # [64, S]
                ebt = ebp.tile([128, S], BF16, tag="eb", name=f"eb{i}")
                nc.sync.dma_start(
                    out=ebt[:, 0 : w1 - w0],
                    in_=ebT[s, kc * 128 : (kc + 1) * 128, w0:w1],
                )
                ps = ps_mm.tile([128, S], F32, tag="mm", name=f"s{i}")
                # scoresT[k, q] = k @ q.T over the alive window only
                for _hb, c0, c1 in _half_ranges(w0, w1):
                    nc.tensor.matmul(
                        ps[:, c0:c1],
                        kT[:, kc * 128 : (kc + 1) * 128],
                        qT[:, c0:c1],
                        start=True,
                        stop=True,
                    )
                et = ep.tile([128, S], BF16, tag="et", name=f"et{i}")
                nc.scalar.activation(
                    et[:, w0:w1], ps[:, w0:w1], mybir.ActivationFunctionType.Exp
                )
                # fold in the additive bias: exp(s+b) = exp(s)*exp(b)  (DVE)
                et2 = e2p.tile([128, S], BF16, tag="e2", name=f"e2_{i}")
                nc.vector.tensor_tensor(
                    et2[:, w0:w1],
                    et[:, w0:w1],
                    ebt[:, 0 : w1 - w0],
                    op=mybir.AluOpType.mult,
                )
                et2s[(s, kc)] = et2

            def get_pos(s):
                if s not in pos_map:
                    # [65, 512] 1-bank output tiles: rows 0..63 = outT,
                    # row 64 = sum of exp over the alive band
                    pos_map[s] = [
                        ps_sm.tile([HD + 1, 512], F32, tag="sm", name=f"po{s}_{k}")
                        for k in range(2)
                    ]
                return pos_map[s]

            def emit_back_dense(s, kc):
                pos = get_pos(s)
                et2 = et2s[(s, kc)]
                for hb in range(2):
                    nc.tensor.matmul(
                        pos[hb][:],
                        v_sb[kc][:, s, :],
                        et2[:, hb * 512 : (hb + 1) * 512],
                        start=(kc == 0),
                        stop=(kc == KC - 1),
                    )
                if kc == KC - 1:
                    for kc2 in range(KC):
                        del et2s[(s, kc2)]

            def emit_back_sparse(s, qc):
                r = slot_r[s]
                pos = get_pos(s)
                hb, c0 = qc // 4, (qc % 4) * 128
                klo, khi = max(0, qc - r), min(KC - 1, qc + r)
                for kc in range(klo, khi + 1):
                    nc.tensor.matmul(
                        pos[hb][:, c0 : c0 + 128],
                        v_sb[kc][:, s, :],
                        et2s[(s, kc)][:, qc * 128 : (qc + 1) * 128],
                        start=(kc == klo),
                        stop=(kc == khi),
                    )
                if qc == KC - 1:
                    for kc in range(KC):
                        del et2s[(s, kc)]

            def emit_tail(s, half):
                # normalize: out[d,q] * (1/sum[q]).  1/sum via
                # reciprocal_approx_fast (18 bits; sums are benign), broadcast
                # along partitions on the idle GpSimd, multiply on DVE.
                if True:
                    p = get_pos(s)[half]
                    # the sum row lives at PSUM partition 64; DVE cannot
                    # read partition 64 into partition 0, ScalarE can
                    smf = op_.tile([1, 512], F32, tag="smf")
                    nc.scalar.activation(
                        smf[:], p[HD : HD + 1, :],
                        mybir.ActivationFunctionType.Copy,
                    )
                    rcf = op_.tile([1, 512], F32, tag="rcf")
                    nc.vector.reciprocal_approx_fast(rcf[:], smf[:])
                    rb = op_.tile([HD, 512], F32, tag="rb")
                    nc.gpsimd.partition_broadcast(rb[:], rcf[:])
                    ot = op_.tile([HD, 512], F32, tag="ot")
                    nc.vector.tensor_tensor(
                        ot[:], p[0:HD, :], rb[:], op=mybir.AluOpType.mult
                    )
                    nc.sync.dma_start(
                        out=oT[
                            s * HD : (s + 1) * HD, half * 512 : (half + 1) * 512
                        ],
                        in_=ot[:],
                    )

            # backs: (emit-at front index, emit fn). Dense back (s,kc) needs
            # front (s,kc); sparse back (s,qc) needs fronts through qc+r.
            # Sparse items are small (short exp/mult chains), so they ride
            # closer behind their fronts. Normalize tails are deferred a
            # couple of items past their last AV so the ScalarE copy / DVE
            # reciprocal never block the in-order engine queues waiting for
            # the AV accumulation to finish.
            backs = []
            for s in range(HPC):
                r = slot_r[s]
                if r >= KC - 1:
                    for kc in range(KC):
                        backs.append(
                            (
                                idx_of[(s, kc)] + 4,
                                lambda s=s, kc=kc: emit_back_dense(s, kc),
                            )
                        )
                    last = idx_of[(s, KC - 1)] + 4
                    backs.append((last + 2, lambda s=s: emit_tail(s, 0)))
                    backs.append((last + 2, lambda s=s: emit_tail(s, 1)))
                else:
                    for qc in range(KC):
                        backs.append(
                            (
                                idx_of[(s, min(KC - 1, qc + r))] + 3,
                                lambda s=s, qc=qc: emit_back_sparse(s, qc),
                            )
                        )
                    backs.append(
                        (idx_of[(s, 3 + r)] + 5, lambda s=s: emit_tail(s, 0))
                    )
                    backs.append(
                        (idx_of[(s, KC - 1)] + 5, lambda s=s: emit_tail(s, 1))
                    )
            backs.sort(key=lambda b: b[0])
            bi = 0
            for i in range(len(items)):
                emit_front(i)
                while bi < len(backs) and backs[bi][0] <= i:
                    backs[bi][1]()
                    bi += 1
            while bi < len(backs):
                backs[bi][1]()
                bi += 1

    # Bacc defers register allocation to its compile() pass, which only runs
    # in finalize(); run_bass_via_pjrt ships the BIR as-is, so finalize here.
    nc.finalize()
    return nc


def shard_inputs(hidden_states, bias, Wqkv_w, Wqkv_b):
    """Slice + lay out the full inputs into 8 per-core input maps."""
    import ml_dtypes

    bf16 = ml_dtypes.bfloat16
    hidden_states = np.asarray(hidden_states, dtype=np.float32)
    bias = np.asarray(bias, dtype=np.float32)
    Wqkv_w = np.asarray(Wqkv_w, dtype=np.float32)
    Wqkv_b = np.asarray(Wqkv_b, dtype=np.float32)

    in_maps = []
    for c in range(N_CORES):
        b, par = c // 2, c % 2
        heads = [PAIRS[s][par] for s in range(HPC)]
        rows = np.concatenate(
            [np.arange(g * D + h * HD, g * D + (h + 1) * HD) for g in range(3) for h in heads]
        )
        wbp2 = np.ascontiguousarray(
            Wqkv_b[rows[0 : 2 * HPC * HD]].reshape(8, 128).T
        ).astype(np.float32)
        wbv2 = np.broadcast_to(
            Wqkv_b[rows[2 * HPC * HD :]].astype(bf16)[None, :], (128, HPC * HD)
        )
        eb = np.exp(bias[b, heads])  # [8, S, S] fp32
        in_maps.append(
            {
                "hw": np.concatenate(
                    [hidden_states[b].T, Wqkv_w[rows].T], axis=1
                ).astype(bf16),
                "wbv": np.ascontiguousarray(wbv2),
                "wbp": wbp2,
                "ebT": np.ascontiguousarray(eb.transpose(0, 2, 1)).astype(bf16),
            }
        )
    return in_maps


def gather(res):
    out = np.empty((B, S, D), dtype=np.float32)
    for c in range(N_CORES):
        b, par = c // 2, c % 2
        for s in range(HPC):
            h = PAIRS[s][par]
            out[b, :, h * HD : (h + 1) * HD] = res.results[c]["oT"][
                s * HD : (s + 1) * HD, :
            ].T
    return out


def sparsity_ok(bias) -> bool:
    """Verify the ACTUAL bias values keep every skipped block below -T_CHECK
    (so its softmax mass is < ~e^-12 of the row total)."""
    if all(r >= KC - 1 for r in SLOT_R):
        return True
    bias = np.asarray(bias, dtype=np.float32)
    bm = bias.reshape(B, H, KC, 128, KC, 128).max(axis=(3, 5))  # [B,H,8,8]
    kc = np.arange(KC)[:, None]
    qc = np.arange(KC)[None, :]
    for s, r in enumerate(SLOT_R):
        dead = np.abs(kc - qc) > r
        if not dead.any():
            continue
        for h in PAIRS[s]:
            if not np.all(bm[:, h][:, dead] < -T_CHECK):
                return False
    return True


_CACHED = {}


def kernel(hidden_states, bias, Wqkv_w, Wqkv_b):
    from concourse.bass_utils import run_bass_kernel_spmd

    slot_r = tuple(SLOT_R if sparsity_ok(bias) else DENSE_R)
    if slot_r not in _CACHED:
        _CACHED[slot_r] = build_bass(list(slot_r))
    in_maps = shard_inputs(hidden_states, bias, Wqkv_w, Wqkv_b)
    res = run_bass_kernel_spmd(
        _CACHED[slot_r], in_maps, core_ids=list(range(N_CORES))
    )
    return gather(res)


# revision 32
# speedup vs baseline: 1.1791x; 1.1791x over previous
"""BertSelfAttention (ALiBi-style additive bias) on 8 TRN2 NeuronCores.

Problem: B=4, S=1024, D=1024, H=16 heads (HD=64), fp32.
  qkv = hidden @ Wqkv_w.T + Wqkv_b
  scores = q @ k.T / sqrt(64) + bias ;  probs = softmax(scores) ; out = probs @ v

Sharding: 8 cores = 4 batches x 2 head-groups of 8 "slots". Core c handles
batch c//2 and takes one head from each of 8 head PAIRS (parity c%2), so both
cores of a batch run the identical program on equal work.

Key ideas vs the naive kernel:
  * exp(s + b) = exp(s) * exp(b): the additive bias never touches the
    TensorEngine. The host ships exp(bias) (bf16); the device multiplies it
    into exp(scores) on the (otherwise idle) DVE. This removes 128 identity
    matmuls (~35us of PE time) per core.
  * ALiBi block sparsity: bias = -slope_h * |q - k|, so (128x128) score
    blocks with bias < -T contribute < e^-T relative softmax mass and are
    skipped entirely (no QK matmul, no exp, no bias DMA, no AV matmul).
    Heads are paired sparse-with-sparse across the two cores of a batch so
    the shared SPMD program uses per-slot block radii = max over the pair.
    kernel() verifies the dead blocks against the ACTUAL runtime bias values
    and falls back to a dense variant of the same program if the input is
    not ALiBi-shaped.
  * Scores are computed transposed (scoresT[k, q]) so the AV matmul
    [v | 1].T @ expT also yields the softmax denominator in row 64;
    normalization = fast reciprocal + partition-broadcast + DVE multiply.
  * No max-subtraction in softmax: |scores| <= ~10 in fp32 cannot overflow,
    and large-negative ALiBi bias underflows exp to a clean 0.
  * DMA order: the 8 hidden/weight chunks are triggered first (constants
    after) so the first matmul starts as soon as chunk 0 lands.
"""

import numpy as np

import concourse.bacc as bacc
import concourse.bass as bass
import concourse.mybir as mybir
from concourse.tile import TileContext

B, S, D = 4, 1024, 1024
H = 16
HD = 64  # head dim
N_CORES = 8
HPC = 8  # head-slots per core
OC = 3 * HPC * HD  # 1536 fused-qkv output rows per core
F32 = mybir.dt.float32
BF16 = mybir.dt.bfloat16

KC = S // 128  # 8 key-token chunks of 128
DC = D // 128  # 8 contraction chunks of 128

# --- ALiBi sparsity geometry -------------------------------------------------
# Head h (0-indexed) has slope 2^(-8(h+1)/16). A (kc, qc) block of 128x128
# token pairs is dead when its *smallest* |q - k| distance, 128*|kc-qc| - 127,
# gives bias below -T_SPARSE everywhere in the block.
T_SPARSE = 6.0
T_CHECK = 5.5  # runtime verification margin for dead blocks
_DENSE_MIN_R = 4  # pair radii above 3 round up to fully dense (r=7)


def _alibi_radii(thresh: float) -> list[int]:
    slopes = 2.0 ** (-8.0 * (np.arange(1, H + 1) / H))
    radii = []
    for sl in slopes:
        r = 7
        while r >= 1 and 128 * r - 127 > thresh / sl:
            r -= 1
        radii.append(r)
    return radii


def _make_slots(radii: list[int]):
    order = sorted(range(H), key=lambda h: (radii[h], h))
    pairs = [(order[2 * i], order[2 * i + 1]) for i in range(HPC)]
    slot_r = [max(radii[a], radii[b]) for a, b in pairs]
    # densest slots first: the tail (last slot's serial normalize chain)
    # then belongs to the sparsest head, and early slots line up with the
    # first-finished qkv bands.
    perm = sorted(range(HPC), key=lambda i: (-slot_r[i], i))
    # PSUM accumulation groups must have a uniform footprint (region-varying
    # start/stop within a bank silently drops contributions), so sparse slots
    # use per-query-block accumulation groups. That only pays off for small
    # radii; near-dense slots round up to the plain dense pattern.
    slot_r = [r if r < _DENSE_MIN_R else KC - 1 for r in slot_r]
    return [pairs[i] for i in perm], [slot_r[i] for i in perm]


PAIRS, SLOT_R = _make_slots(_alibi_radii(T_SPARSE))
DENSE_R = [7] * HPC


def _window(r: int, kc: int) -> tuple[int, int]:
    """Alive query-block range [lo, hi] (inclusive) for key-chunk kc."""
    return max(0, kc - r), min(KC - 1, kc + r)


def _half_ranges(w0: int, w1: int):
    """Split column range [w0, w1) at the 512 PSUM-bank boundary."""
    out = []
    for hb in range(2):
        c0, c1 = max(w0, hb * 512), min(w1, (hb + 1) * 512)
        if c0 < c1:
            out.append((hb, c0, c1))
    return out


def build_bass(slot_r=None) -> bass.Bass:
    if slot_r is None:
        slot_r = SLOT_R
    nc = bacc.Bacc()

    hw = nc.declare_dram_parameter("hw", [D, S + OC], BF16, isOutput=False)
    wbv = nc.declare_dram_parameter("wbv", [128, HPC * HD], BF16, isOutput=False)
    wbp = nc.declare_dram_parameter("wbp", [128, 8], F32, isOutput=False)
    ebT = nc.declare_dram_parameter("ebT", [HPC, S, S], BF16, isOutput=False)
    oT = nc.declare_dram_parameter("oT", [HPC * HD, S], F32, isOutput=True)

    with TileContext(nc) as tc:
        with (
            tc.tile_pool(name="const", bufs=1) as constp,
            tc.tile_pool(name="weights", bufs=1) as wp,
            tc.tile_pool(name="qk", bufs=1) as qkp,
            tc.tile_pool(name="vex", bufs=1) as vp,
            tc.tile_pool(name="ebias", bufs=8) as ebp,
            tc.tile_pool(name="exp", bufs=5) as ep,
            # sparse qc-major AV reads et2(kc) until qc = kc + 2r + LAG,
            # so keep enough rotation depth for r=3 slots
            tc.tile_pool(name="exp2", bufs=12) as e2p,
            tc.tile_pool(name="outs", bufs=2) as op_,
            tc.tile_pool(name="ps_mm", bufs=2, space="PSUM") as ps_mm,
            tc.tile_pool(name="ps_sm", bufs=4, space="PSUM") as ps_sm,
        ):
            # --- stage inputs ---------------------------------------------
            # hidden^T | W^T chunks first: the first matmul only waits on
            # chunk 0. Small constants ride behind them on the queue.
            hT_sb = []
            wT_sb = []
            for c in range(DC):
                hwt = wp.tile([128, S + OC], BF16, tag=f"hw{c}", name=f"hw{c}")
                nc.sync.dma_start(out=hwt[:], in_=hw[c * 128 : (c + 1) * 128, :])
                hT_sb.append(hwt[:, 0:S])
                wT_sb.append(hwt[:, S : S + OC])

            wbv_sb = constp.tile([128, HPC, HD], BF16)
            nc.sync.dma_start(
                out=wbv_sb[:].rearrange("p h d -> p (h d)"), in_=wbv[:]
            )
            wbp_sb = constp.tile([128, 8], F32)
            nc.sync.dma_start(out=wbp_sb[:], in_=wbp[:])

            # --- phase 1: fused QKV projection -----------------------------
            # qk_sb[j][p, t]: j in 0..3 -> q rows (pre-scaled by 1/8),
            #                 j in 4..7 -> k rows. Row (j%4)*128+p = oc index.
            qk_sb = [
                qkp.tile([128, S], BF16, tag=f"qk{j}", name=f"qk{j}")
                for j in range(8)
            ]
            # v_sb[t][p, s, 0:64] = v slot s, token t*128+p; [.., 64] = 1.0
            v_sb = [
                vp.tile([128, HPC, HD + 1], BF16, tag=f"vx{t}", name=f"v{t}")
                for t in range(KC)
            ]

            def qk_blk(j):
                ps = ps_mm.tile([128, S], F32, tag="mm", name=f"qkp{j}")

                def mm(c):
                    lw = wT_sb[c][:, j * 128 : (j + 1) * 128]
                    for half in range(2):
                        nc.tensor.matmul(
                            ps[:, half * 512 : (half + 1) * 512],
                            lw,
                            hT_sb[c][:, half * 512 : (half + 1) * 512],
                            start=(c == 0),
                            stop=(c == DC - 1),
                        )

                def fin():
                    # copy to SBUF, adding the per-partition qkv bias and
                    # folding the 1/sqrt(HD) score scale into q rows (DVE)
                    if j < 4:
                        nc.vector.tensor_scalar(
                            qk_sb[j][:], ps[:], wbp_sb[:, j : j + 1], 0.125,
                            op0=mybir.AluOpType.add, op1=mybir.AluOpType.mult,
                        )
                    else:
                        nc.vector.tensor_scalar_add(
                            qk_sb[j][:], ps[:], wbp_sb[:, j : j + 1]
                        )

                return mm, fin

            def v_blk(t):
                ps = ps_sm.tile([128, HPC * HD], F32, tag="sm", name=f"vps{t}")

                def mm(c):
                    nc.tensor.matmul(
                        ps[:],
                        hT_sb[c][:, t * 128 : (t + 1) * 128],
                        wT_sb[c][:, 2 * HPC * HD : 3 * HPC * HD],
                        start=(c == 0),
                        stop=(c == DC - 1),
                    )

                def fin():
                    nc.vector.tensor_tensor(
                        v_sb[t][:, :, 0:HD],
                        ps[:].rearrange("p (h d) -> p h d", h=HPC),
                        wbv_sb[:],
                        op=mybir.AluOpType.add,
                    )
                    nc.scalar.activation(
                        v_sb[t][:, :, HD : HD + 1],
                        v_sb[t][:, :, 0:1],
                        mybir.ActivationFunctionType.Identity,
                        scale=0.0,
                        bias=1.0,
                    )

                return mm, fin

            bands = [
                [qk_blk(0), qk_blk(4), v_blk(0), v_blk(1), v_blk(2)],
                [qk_blk(1), qk_blk(5), v_blk(3), v_blk(4), v_blk(5)],
                [qk_blk(2), qk_blk(6), v_blk(6), v_blk(7)],
                [qk_blk(3), qk_blk(7)],
            ]
            for band in bands:
                for c in range(DC):
                    for mm, _ in band:
                        mm(c)
                for _, fin in band:
                    fin()

            # --- phase 2: attention ----------------------------------------
            # Software-pipelined across (slot, k-chunk) items: the AV matmuls
            # are emitted LAG items late so the in-order PE stream never
            # stalls waiting on an item's exp * exp(bias).
            #
            # Dense slots (r=7) accumulate kc-major with full-half groups.
            # Sparse slots accumulate qc-major: per query-block, one
            # uniform-footprint PSUM group over its alive key-chunks
            # (region-varying start/stop within a group loses contributions).
            # Items are slot-sequential except the last two (sparsest) slots,
            # which interleave so their end-of-kernel drains overlap.
            items = [(s, kc) for s in range(HPC - 2) for kc in range(KC)]
            for kc in range(KC):
                items.append((HPC - 2, kc))
                items.append((HPC - 1, kc))
            idx_of = {it: i for i, it in enumerate(items)}
            et2s: dict[tuple, object] = {}
            pos_map: dict[int, list] = {}

            def emit_front(i):
                s, kc = items[i]
                r = slot_r[s]
                lo, hi = _window(r, kc)
                w0, w1 = lo * 128, (hi + 1) * 128
                j, po = s // 2, (s % 2) * 64
                qT = qk_sb[j][po : po + 64, :]  # [64, S] (already /8)
                kT = qk_sb[4 + j][po : po + 64, :]  # [64, S]
                ebt = ebp.tile([128, S], BF16, tag="eb", name=f"eb{i}")
                nc.sync.dma_start(
                    out=ebt[:, 0 : w1 - w0],
                    in_=ebT[s, kc * 128 : (kc + 1) * 128, w0:w1],
                )
                ps = ps_mm.tile([128, S], F32, tag="mm", name=f"s{i}")
                # scoresT[k, q] = k @ q.T over the alive window only
                for _hb, c0, c1 in _half_ranges(w0, w1):
                    nc.tensor.matmul(
                        ps[:, c0:c1],
                        kT[:, kc * 128 : (kc + 1) * 128],
                        qT[:, c0:c1],
                        start=True,
                        stop=True,
                    )
                et = ep.tile([128, S], BF16, tag="et", name=f"et{i}")
                nc.scalar.activation(
                    et[:, w0:w1], ps[:, w0:w1], mybir.ActivationFunctionType.Exp
                )
                # fold in the additive bias: exp(s+b) = exp(s)*exp(b)  (DVE)
                et2 = e2p.tile([128, S], BF16, tag="e2", name=f"e2_{i}")
                nc.vector.tensor_tensor(
                    et2[:, w0:w1],
                    et[:, w0:w1],
                    ebt[:, 0 : w1 - w0],
                    op=mybir.AluOpType.mult,
                )
                et2s[(s, kc)] = et2

            def get_pos(s):
                if s not in pos_map:
                    # [65, 512] 1-bank output tiles: rows 0..63 = outT,
                    # row 64 = sum of exp over the alive band
                    pos_map[s] = [
                        ps_sm.tile([HD + 1, 512], F32, tag="sm", name=f"po{s}_{k}")
                        for k in range(2)
                    ]
                return pos_map[s]

            def emit_back_dense(s, kc):
                pos = get_pos(s)
                et2 = et2s[(s, kc)]
                for hb in range(2):
                    nc.tensor.matmul(
                        pos[hb][:],
                        v_sb[kc][:, s, :],
                        et2[:, hb * 512 : (hb + 1) * 512],
                        start=(kc == 0),
                        stop=(kc == KC - 1),
                    )
                if kc == KC - 1:
                    for kc2 in range(KC):
                        del et2s[(s, kc2)]
                    emit_tail(s, 0)
                    emit_tail(s, 1)

            def emit_back_sparse(s, qc):
                r = slot_r[s]
                pos = get_pos(s)
                hb, c0 = qc // 4, (qc % 4) * 128
                klo, khi = max(0, qc - r), min(KC - 1, qc + r)
                for kc in range(klo, khi + 1):
                    nc.tensor.matmul(
                        pos[hb][:, c0 : c0 + 128],
                        v_sb[kc][:, s, :],
                        et2s[(s, kc)][:, qc * 128 : (qc + 1) * 128],
                        start=(kc == klo),
                        stop=(kc == khi),
                    )
                if qc == 3:
                    # queries 0..511 complete: normalize half 0 early
                    emit_tail(s, 0)
                if qc == KC - 1:
                    for kc in range(KC):
                        del et2s[(s, kc)]
                    emit_tail(s, 1)

            def emit_tail(s, half):
                # normalize: out[d,q] * (1/sum[q]).  1/sum via
                # reciprocal_approx_fast (18 bits; sums are benign), broadcast
                # along partitions on the idle GpSimd, multiply on DVE.
                if True:
                    p = get_pos(s)[half]
                    # the sum row lives at PSUM partition 64; DVE cannot
                    # read partition 64 into partition 0, ScalarE can
                    smf = op_.tile([1, 512], F32, tag="smf")
                    nc.scalar.activation(
                        smf[:], p[HD : HD + 1, :],
                        mybir.ActivationFunctionType.Copy,
                    )
                    rcf = op_.tile([1, 512], F32, tag="rcf")
                    nc.vector.reciprocal_approx_fast(rcf[:], smf[:])
                    rb = op_.tile([HD, 512], F32, tag="rb")
                    nc.gpsimd.partition_broadcast(rb[:], rcf[:])
                    ot = op_.tile([HD, 512], F32, tag="ot")
                    nc.vector.tensor_tensor(
                        ot[:], p[0:HD, :], rb[:], op=mybir.AluOpType.mult
                    )
                    nc.sync.dma_start(
                        out=oT[
                            s * HD : (s + 1) * HD, half * 512 : (half + 1) * 512
                        ],
                        in_=ot[:],
                    )

            # backs: (emit-at front index, emit fn). Dense back (s,kc) needs
            # front (s,kc); sparse back (s,qc) needs fronts through qc+r.
            # Sparse items are small (short exp/mult chains), so they ride
            # closer behind their fronts. Normalize tails are deferred a
            # couple of items past their last AV so the ScalarE copy / DVE
            # reciprocal never block the in-order engine queues waiting for
            # the AV accumulation to finish.
            backs = []
            for s in range(HPC):
                r = slot_r[s]
                if r >= KC - 1:
                    for kc in range(KC):
                        backs.append(
                            (
                                idx_of[(s, kc)] + 4,
                                lambda s=s, kc=kc: emit_back_dense(s, kc),
                            )
                        )
                else:
                    for qc in range(KC):
                        backs.append(
                            (
                                idx_of[(s, min(KC - 1, qc + r))] + 3,
                                lambda s=s, qc=qc: emit_back_sparse(s, qc),
                            )
                        )
            backs.sort(key=lambda b: b[0])
            bi = 0
            for i in range(len(items)):
                emit_front(i)
                while bi < len(backs) and backs[bi][0] <= i:
                    backs[bi][1]()
                    bi += 1
            while bi < len(backs):
                backs[bi][1]()
                bi += 1

    # Bacc defers register allocation to its compile() pass, which only runs
    # in finalize(); run_bass_via_pjrt ships the BIR as-is, so finalize here.
    nc.finalize()
    return nc


def shard_inputs(hidden_states, bias, Wqkv_w, Wqkv_b):
    """Slice + lay out the full inputs into 8 per-core input maps."""
    import ml_dtypes

    bf16 = ml_dtypes.bfloat16
    hidden_states = np.asarray(hidden_states, dtype=np.float32)
    bias = np.asarray(bias, dtype=np.float32)
    Wqkv_w = np.asarray(Wqkv_w, dtype=np.float32)
    Wqkv_b = np.asarray(Wqkv_b, dtype=np.float32)

    in_maps = []
    for c in range(N_CORES):
        b, par = c // 2, c % 2
        heads = [PAIRS[s][par] for s in range(HPC)]
        rows = np.concatenate(
            [np.arange(g * D + h * HD, g * D + (h + 1) * HD) for g in range(3) for h in heads]
        )
        wbp2 = np.ascontiguousarray(
            Wqkv_b[rows[0 : 2 * HPC * HD]].reshape(8, 128).T
        ).astype(np.float32)
        wbv2 = np.broadcast_to(
            Wqkv_b[rows[2 * HPC * HD :]].astype(bf16)[None, :], (128, HPC * HD)
        )
        eb = np.exp(bias[b, heads])  # [8, S, S] fp32
        in_maps.append(
            {
                "hw": np.concatenate(
                    [hidden_states[b].T, Wqkv_w[rows].T], axis=1
                ).astype(bf16),
                "wbv": np.ascontiguousarray(wbv2),
                "wbp": wbp2,
                "ebT": np.ascontiguousarray(eb.transpose(0, 2, 1)).astype(bf16),
            }
        )
    return in_maps


def gather(res):
    out = np.empty((B, S, D), dtype=np.float32)
    for c in range(N_CORES):
        b, par = c // 2, c % 2
        for s in range(HPC):
            h = PAIRS[s][par]
            out[b, :, h * HD : (h + 1) * HD] = res.results[c]["oT"][
                s * HD : (s + 1) * HD, :
            ].T
    return out


def sparsity_ok(bias) -> bool:
    """Verify the ACTUAL bias values keep every skipped block below -T_CHECK
    (so its softmax mass is < ~e^-12 of the row total)."""
    if all(r >= KC - 1 for r in SLOT_R):
        return True
    bias = np.asarray(bias, dtype=np.float32)
    bm = bias.reshape(B, H, KC, 128, KC, 128).max(axis=(3, 5))  # [B,H,8,8]
    kc = np.arange(KC)[:, None]
    qc = np.arange(KC)[None, :]
    for s, r in enumerate(SLOT_R):
        dead = np.abs(kc - qc) > r
        if not dead.any():
            continue
        for h in PAIRS[s]:
            if not np.all(bm[:, h][:, dead] < -T_CHECK):
                return False
    return True


_CACHED = {}


def kernel(hidden_states, bias, Wqkv_w, Wqkv_b):
    from concourse.bass_utils import run_bass_kernel_spmd

    slot_r = tuple(SLOT_R if sparsity_ok(bias) else DENSE_R)
    if slot_r not in _CACHED:
        _CACHED[slot_r] = build_bass(list(slot_r))
    in_maps = shard_inputs(hidden_states, bias, Wqkv_w, Wqkv_b)
    res = run_bass_kernel_spmd(
        _CACHED[slot_r], in_maps, core_ids=list(range(N_CORES))
    )
    return gather(res)


# revision 33
# speedup vs baseline: 1.2003x; 1.0181x over previous
"""BertSelfAttention (ALiBi-style additive bias) on 8 TRN2 NeuronCores.

Problem: B=4, S=1024, D=1024, H=16 heads (HD=64), fp32.
  qkv = hidden @ Wqkv_w.T + Wqkv_b
  scores = q @ k.T / sqrt(64) + bias ;  probs = softmax(scores) ; out = probs @ v

Sharding: 8 cores = 4 batches x 2 head-groups of 8 "slots". Core c handles
batch c//2 and takes one head from each of 8 head PAIRS (parity c%2), so both
cores of a batch run the identical program on equal work.

Key ideas vs the naive kernel:
  * exp(s + b) = exp(s) * exp(b): the additive bias never touches the
    TensorEngine. The host ships exp(bias) (bf16); the device multiplies it
    into exp(scores) on the (otherwise idle) DVE. This removes 128 identity
    matmuls (~35us of PE time) per core.
  * ALiBi block sparsity: bias = -slope_h * |q - k|, so (128x128) score
    blocks with bias < -T contribute < e^-T relative softmax mass and are
    skipped entirely (no QK matmul, no exp, no bias DMA, no AV matmul).
    Heads are paired sparse-with-sparse across the two cores of a batch so
    the shared SPMD program uses per-slot block radii = max over the pair.
    kernel() verifies the dead blocks against the ACTUAL runtime bias values
    and falls back to a dense variant of the same program if the input is
    not ALiBi-shaped.
  * Scores are computed transposed (scoresT[k, q]) so the AV matmul
    [v | 1].T @ expT also yields the softmax denominator in row 64;
    normalization = fast reciprocal + partition-broadcast + DVE multiply.
  * No max-subtraction in softmax: |scores| <= ~10 in fp32 cannot overflow,
    and large-negative ALiBi bias underflows exp to a clean 0.
  * DMA order: the 8 hidden/weight chunks are triggered first (constants
    after) so the first matmul starts as soon as chunk 0 lands.
"""

import numpy as np

import concourse.bacc as bacc
import concourse.bass as bass
import concourse.mybir as mybir
from concourse.tile import TileContext

B, S, D = 4, 1024, 1024
H = 16
HD = 64  # head dim
N_CORES = 8
HPC = 8  # head-slots per core
OC = 3 * HPC * HD  # 1536 fused-qkv output rows per core
F32 = mybir.dt.float32
BF16 = mybir.dt.bfloat16

KC = S // 128  # 8 key-token chunks of 128
DC = D // 128  # 8 contraction chunks of 128

# --- ALiBi sparsity geometry -------------------------------------------------
# Head h (0-indexed) has slope 2^(-8(h+1)/16). A (kc, qc) block of 128x128
# token pairs is dead when its *smallest* |q - k| distance, 128*|kc-qc| - 127,
# gives bias below -T_SPARSE everywhere in the block.
T_SPARSE = 6.0
T_CHECK = 5.5  # runtime verification margin for dead blocks
_DENSE_MIN_R = 4  # pair radii above 3 round up to fully dense (r=7)


def _alibi_radii(thresh: float) -> list[int]:
    slopes = 2.0 ** (-8.0 * (np.arange(1, H + 1) / H))
    radii = []
    for sl in slopes:
        r = 7
        while r >= 1 and 128 * r - 127 > thresh / sl:
            r -= 1
        radii.append(r)
    return radii


def _make_slots(radii: list[int]):
    order = sorted(range(H), key=lambda h: (radii[h], h))
    pairs = [(order[2 * i], order[2 * i + 1]) for i in range(HPC)]
    slot_r = [max(radii[a], radii[b]) for a, b in pairs]
    # densest slots first: the tail (last slot's serial normalize chain)
    # then belongs to the sparsest head, and early slots line up with the
    # first-finished qkv bands.
    perm = sorted(range(HPC), key=lambda i: (-slot_r[i], i))
    # PSUM accumulation groups must have a uniform footprint (region-varying
    # start/stop within a bank silently drops contributions), so sparse slots
    # use per-query-block accumulation groups. That only pays off for small
    # radii; near-dense slots round up to the plain dense pattern.
    slot_r = [r if r < _DENSE_MIN_R else KC - 1 for r in slot_r]
    return [pairs[i] for i in perm], [slot_r[i] for i in perm]


PAIRS, SLOT_R = _make_slots(_alibi_radii(T_SPARSE))
DENSE_R = [7] * HPC


def _window(r: int, kc: int) -> tuple[int, int]:
    """Alive query-block range [lo, hi] (inclusive) for key-chunk kc."""
    return max(0, kc - r), min(KC - 1, kc + r)


def _half_ranges(w0: int, w1: int):
    """Split column range [w0, w1) at the 512 PSUM-bank boundary."""
    out = []
    for hb in range(2):
        c0, c1 = max(w0, hb * 512), min(w1, (hb + 1) * 512)
        if c0 < c1:
            out.append((hb, c0, c1))
    return out


def build_bass(slot_r=None) -> bass.Bass:
    if slot_r is None:
        slot_r = SLOT_R
    nc = bacc.Bacc()

    hw = nc.declare_dram_parameter("hw", [D, S + OC], BF16, isOutput=False)
    wbv = nc.declare_dram_parameter("wbv", [128, HPC * HD], BF16, isOutput=False)
    wbp = nc.declare_dram_parameter("wbp", [128, 8], F32, isOutput=False)
    ebT = nc.declare_dram_parameter("ebT", [HPC, S, S], BF16, isOutput=False)
    oT = nc.declare_dram_parameter("oT", [HPC * HD, S], F32, isOutput=True)

    with TileContext(nc) as tc:
        with (
            tc.tile_pool(name="const", bufs=1) as constp,
            tc.tile_pool(name="weights", bufs=1) as wp,
            tc.tile_pool(name="qk", bufs=1) as qkp,
            tc.tile_pool(name="vex", bufs=1) as vp,
            tc.tile_pool(name="ebias", bufs=8) as ebp,
            tc.tile_pool(name="exp", bufs=5) as ep,
            # sparse qc-major AV reads et2(kc) until qc = kc + 2r + LAG,
            # so keep enough rotation depth for r=3 slots
            tc.tile_pool(name="exp2", bufs=12) as e2p,
            tc.tile_pool(name="outs", bufs=2) as op_,
            tc.tile_pool(name="ps_mm", bufs=2, space="PSUM") as ps_mm,
            tc.tile_pool(name="ps_sm", bufs=4, space="PSUM") as ps_sm,
        ):
            # --- stage inputs ---------------------------------------------
            # hidden^T | W^T chunks first: the first matmul only waits on
            # chunk 0. Small constants ride behind them on the queue.
            hT_sb = []
            wT_sb = []
            for c in range(DC):
                hwt = wp.tile([128, S + OC], BF16, tag=f"hw{c}", name=f"hw{c}")
                nc.sync.dma_start(out=hwt[:], in_=hw[c * 128 : (c + 1) * 128, :])
                hT_sb.append(hwt[:, 0:S])
                wT_sb.append(hwt[:, S : S + OC])

            wbv_sb = constp.tile([128, HPC, HD], BF16)
            nc.sync.dma_start(
                out=wbv_sb[:].rearrange("p h d -> p (h d)"), in_=wbv[:]
            )
            wbp_sb = constp.tile([128, 8], F32)
            nc.sync.dma_start(out=wbp_sb[:], in_=wbp[:])

            # --- phase 1: fused QKV projection -----------------------------
            # qk_sb[j][p, t]: j in 0..3 -> q rows (pre-scaled by 1/8),
            #                 j in 4..7 -> k rows. Row (j%4)*128+p = oc index.
            qk_sb = [
                qkp.tile([128, S], BF16, tag=f"qk{j}", name=f"qk{j}")
                for j in range(8)
            ]
            # v_sb[t][p, s, 0:64] = v slot s, token t*128+p; [.., 64] = 1.0
            v_sb = [
                vp.tile([128, HPC, HD + 1], BF16, tag=f"vx{t}", name=f"v{t}")
                for t in range(KC)
            ]

            def qk_blk(j):
                ps = ps_mm.tile([128, S], F32, tag="mm", name=f"qkp{j}")

                def mm(c):
                    lw = wT_sb[c][:, j * 128 : (j + 1) * 128]
                    for half in range(2):
                        nc.tensor.matmul(
                            ps[:, half * 512 : (half + 1) * 512],
                            lw,
                            hT_sb[c][:, half * 512 : (half + 1) * 512],
                            start=(c == 0),
                            stop=(c == DC - 1),
                        )

                def fin():
                    # copy to SBUF, adding the per-partition qkv bias and
                    # folding the 1/sqrt(HD) score scale into q rows (DVE)
                    if j < 4:
                        nc.vector.tensor_scalar(
                            qk_sb[j][:], ps[:], wbp_sb[:, j : j + 1], 0.125,
                            op0=mybir.AluOpType.add, op1=mybir.AluOpType.mult,
                        )
                    else:
                        nc.vector.tensor_scalar_add(
                            qk_sb[j][:], ps[:], wbp_sb[:, j : j + 1]
                        )

                return mm, fin

            def v_blk(t):
                ps = ps_sm.tile([128, HPC * HD], F32, tag="sm", name=f"vps{t}")

                def mm(c):
                    nc.tensor.matmul(
                        ps[:],
                        hT_sb[c][:, t * 128 : (t + 1) * 128],
                        wT_sb[c][:, 2 * HPC * HD : 3 * HPC * HD],
                        start=(c == 0),
                        stop=(c == DC - 1),
                    )

                def fin():
                    nc.vector.tensor_tensor(
                        v_sb[t][:, :, 0:HD],
                        ps[:].rearrange("p (h d) -> p h d", h=HPC),
                        wbv_sb[:],
                        op=mybir.AluOpType.add,
                    )
                    nc.scalar.activation(
                        v_sb[t][:, :, HD : HD + 1],
                        v_sb[t][:, :, 0:1],
                        mybir.ActivationFunctionType.Identity,
                        scale=0.0,
                        bias=1.0,
                    )

                return mm, fin

            bands = [
                [qk_blk(0), qk_blk(4), v_blk(0), v_blk(1), v_blk(2)],
                [qk_blk(1), qk_blk(5), v_blk(3), v_blk(4), v_blk(5)],
                [qk_blk(2), qk_blk(6), v_blk(6), v_blk(7)],
                [qk_blk(3), qk_blk(7)],
            ]
            for band in bands:
                for c in range(DC):
                    for mm, _ in band:
                        mm(c)
                for _, fin in band:
                    fin()

            # --- phase 2: attention ----------------------------------------
            # Software-pipelined across (slot, k-chunk) items: the AV matmuls
            # are emitted LAG items late so the in-order PE stream never
            # stalls waiting on an item's exp * exp(bias).
            #
            # Dense slots (r=7) accumulate kc-major with full-half groups.
            # Sparse slots accumulate qc-major: per query-block, one
            # uniform-footprint PSUM group over its alive key-chunks
            # (region-varying start/stop within a group loses contributions).
            # Slots are processed in interleaved PAIRS (a,b): [a.kc0, b.kc0,
            # a.kc1, ...]. Pairing a ScalarE-heavy dense slot with a PE-cheap
            # sparse slot keeps both engines below the pipeline rate, and a
            # pair's two pos tiles exactly fill the 4-buffer PSUM rotation.
            order = sorted(range(HPC), key=lambda s: (-slot_r[s], s))
            pairs2 = [(order[i], order[HPC - 1 - i]) for i in range(HPC // 2)]
            items = []
            for a, b in pairs2:
                for kc in range(KC):
                    items.append((a, kc))
                    items.append((b, kc))
            idx_of = {it: i for i, it in enumerate(items)}
            et2s: dict[tuple, object] = {}
            pos_map: dict[int, list] = {}

            def emit_front(i):
                s, kc = items[i]
                r = slot_r[s]
                lo, hi = _window(r, kc)
                w0, w1 = lo * 128, (hi + 1) * 128
                j, po = s // 2, (s % 2) * 64
                qT = qk_sb[j][po : po + 64, :]  # [64, S] (already /8)
                kT = qk_sb[4 + j][po : po + 64, :]  # [64, S]
                ebt = ebp.tile([128, S], BF16, tag="eb", name=f"eb{i}")
                nc.sync.dma_start(
                    out=ebt[:, 0 : w1 - w0],
                    in_=ebT[s, kc * 128 : (kc + 1) * 128, w0:w1],
                )
                ps = ps_mm.tile([128, S], F32, tag="mm", name=f"s{i}")
                # scoresT[k, q] = k @ q.T over the alive window only
                for _hb, c0, c1 in _half_ranges(w0, w1):
                    nc.tensor.matmul(
                        ps[:, c0:c1],
                        kT[:, kc * 128 : (kc + 1) * 128],
                        qT[:, c0:c1],
                        start=True,
                        stop=True,
                    )
                et = ep.tile([128, S], BF16, tag="et", name=f"et{i}")
                nc.scalar.activation(
                    et[:, w0:w1], ps[:, w0:w1], mybir.ActivationFunctionType.Exp
                )
                # fold in the additive bias: exp(s+b) = exp(s)*exp(b)  (DVE)
                et2 = e2p.tile([128, S], BF16, tag="e2", name=f"e2_{i}")
                nc.vector.tensor_tensor(
                    et2[:, w0:w1],
                    et[:, w0:w1],
                    ebt[:, 0 : w1 - w0],
                    op=mybir.AluOpType.mult,
                )
                et2s[(s, kc)] = et2

            def get_pos(s):
                if s not in pos_map:
                    # [65, 512] 1-bank output tiles: rows 0..63 = outT,
                    # row 64 = sum of exp over the alive band
                    pos_map[s] = [
                        ps_sm.tile([HD + 1, 512], F32, tag="sm", name=f"po{s}_{k}")
                        for k in range(2)
                    ]
                return pos_map[s]

            def emit_back_dense(s, kc):
                pos = get_pos(s)
                et2 = et2s[(s, kc)]
                for hb in range(2):
                    nc.tensor.matmul(
                        pos[hb][:],
                        v_sb[kc][:, s, :],
                        et2[:, hb * 512 : (hb + 1) * 512],
                        start=(kc == 0),
                        stop=(kc == KC - 1),
                    )
                if kc == KC - 1:
                    for kc2 in range(KC):
                        del et2s[(s, kc2)]
                    emit_tail(s, 0)
                    emit_tail(s, 1)

            def emit_back_sparse(s, qc):
                r = slot_r[s]
                pos = get_pos(s)
                hb, c0 = qc // 4, (qc % 4) * 128
                klo, khi = max(0, qc - r), min(KC - 1, qc + r)
                for kc in range(klo, khi + 1):
                    nc.tensor.matmul(
                        pos[hb][:, c0 : c0 + 128],
                        v_sb[kc][:, s, :],
                        et2s[(s, kc)][:, qc * 128 : (qc + 1) * 128],
                        start=(kc == klo),
                        stop=(kc == khi),
                    )
                if qc == 3:
                    # queries 0..511 complete: normalize half 0 early
                    emit_tail(s, 0)
                if qc == KC - 1:
                    for kc in range(KC):
                        del et2s[(s, kc)]
                    emit_tail(s, 1)

            def emit_tail(s, half):
                # normalize: out[d,q] * (1/sum[q]).  1/sum via
                # reciprocal_approx_fast (18 bits; sums are benign), broadcast
                # along partitions on the idle GpSimd, multiply on DVE.
                if True:
                    p = get_pos(s)[half]
                    # the sum row lives at PSUM partition 64; DVE cannot
                    # read partition 64 into partition 0, ScalarE can
                    smf = op_.tile([1, 512], F32, tag="smf")
                    nc.scalar.activation(
                        smf[:], p[HD : HD + 1, :],
                        mybir.ActivationFunctionType.Copy,
                    )
                    rcf = op_.tile([1, 512], F32, tag="rcf")
                    nc.vector.reciprocal_approx_fast(rcf[:], smf[:])
                    rb = op_.tile([HD, 512], F32, tag="rb")
                    nc.gpsimd.partition_broadcast(rb[:], rcf[:])
                    ot = op_.tile([HD, 512], F32, tag="ot")
                    nc.vector.tensor_tensor(
                        ot[:], p[0:HD, :], rb[:], op=mybir.AluOpType.mult
                    )
                    nc.sync.dma_start(
                        out=oT[
                            s * HD : (s + 1) * HD, half * 512 : (half + 1) * 512
                        ],
                        in_=ot[:],
                    )

            # backs: (emit-at front index, emit fn). Dense back (s,kc) needs
            # front (s,kc); sparse back (s,qc) needs fronts through qc+r.
            # Sparse items are small (short exp/mult chains), so they ride
            # closer behind their fronts. Normalize tails are deferred a
            # couple of items past their last AV so the ScalarE copy / DVE
            # reciprocal never block the in-order engine queues waiting for
            # the AV accumulation to finish.
            backs = []
            for s in range(HPC):
                r = slot_r[s]
                if r >= KC - 1:
                    for kc in range(KC):
                        backs.append(
                            (
                                idx_of[(s, kc)] + 4,
                                lambda s=s, kc=kc: emit_back_dense(s, kc),
                            )
                        )
                else:
                    for qc in range(KC):
                        backs.append(
                            (
                                idx_of[(s, min(KC - 1, qc + r))] + 3,
                                lambda s=s, qc=qc: emit_back_sparse(s, qc),
                            )
                        )
            backs.sort(key=lambda b: b[0])
            bi = 0
            for i in range(len(items)):
                emit_front(i)
                while bi < len(backs) and backs[bi][0] <= i:
                    backs[bi][1]()
                    bi += 1
            while bi < len(backs):
                backs[bi][1]()
                bi += 1

    # Bacc defers register allocation to its compile() pass, which only runs
    # in finalize(); run_bass_via_pjrt ships the BIR as-is, so finalize here.
    nc.finalize()
    return nc


def shard_inputs(hidden_states, bias, Wqkv_w, Wqkv_b):
    """Slice + lay out the full inputs into 8 per-core input maps."""
    import ml_dtypes

    bf16 = ml_dtypes.bfloat16
    hidden_states = np.asarray(hidden_states, dtype=np.float32)
    bias = np.asarray(bias, dtype=np.float32)
    Wqkv_w = np.asarray(Wqkv_w, dtype=np.float32)
    Wqkv_b = np.asarray(Wqkv_b, dtype=np.float32)

    in_maps = []
    for c in range(N_CORES):
        b, par = c // 2, c % 2
        heads = [PAIRS[s][par] for s in range(HPC)]
        rows = np.concatenate(
            [np.arange(g * D + h * HD, g * D + (h + 1) * HD) for g in range(3) for h in heads]
        )
        wbp2 = np.ascontiguousarray(
            Wqkv_b[rows[0 : 2 * HPC * HD]].reshape(8, 128).T
        ).astype(np.float32)
        wbv2 = np.broadcast_to(
            Wqkv_b[rows[2 * HPC * HD :]].astype(bf16)[None, :], (128, HPC * HD)
        )
        eb = np.exp(bias[b, heads])  # [8, S, S] fp32
        in_maps.append(
            {
                "hw": np.concatenate(
                    [hidden_states[b].T, Wqkv_w[rows].T], axis=1
                ).astype(bf16),
                "wbv": np.ascontiguousarray(wbv2),
                "wbp": wbp2,
                "ebT": np.ascontiguousarray(eb.transpose(0, 2, 1)).astype(bf16),
            }
        )
    return in_maps


def gather(res):
    out = np.empty((B, S, D), dtype=np.float32)
    for c in range(N_CORES):
        b, par = c // 2, c % 2
        for s in range(HPC):
            h = PAIRS[s][par]
            out[b, :, h * HD : (h + 1) * HD] = res.results[c]["oT"][
                s * HD : (s + 1) * HD, :
            ].T
    return out


def sparsity_ok(bias) -> bool:
    """Verify the ACTUAL bias values keep every skipped block below -T_CHECK
    (so its softmax mass is < ~e^-12 of the row total)."""
    if all(r >= KC - 1 for r in SLOT_R):
        return True
    bias = np.asarray(bias, dtype=np.float32)
    bm = bias.reshape(B, H, KC, 128, KC, 128).max(axis=(3, 5))  # [B,H,8,8]
    kc = np.arange(KC)[:, None]
    qc = np.arange(KC)[None, :]
    for s, r in enumerate(SLOT_R):
        dead = np.abs(kc - qc) > r
        if not dead.any():
            continue
        for h in PAIRS[s]:
            if not np.all(bm[:, h][:, dead] < -T_CHECK):
                return False
    return True


_CACHED = {}


def kernel(hidden_states, bias, Wqkv_w, Wqkv_b):
    from concourse.bass_utils import run_bass_kernel_spmd

    slot_r = tuple(SLOT_R if sparsity_ok(bias) else DENSE_R)
    if slot_r not in _CACHED:
        _CACHED[slot_r] = build_bass(list(slot_r))
    in_maps = shard_inputs(hidden_states, bias, Wqkv_w, Wqkv_b)
    res = run_bass_kernel_spmd(
        _CACHED[slot_r], in_maps, core_ids=list(range(N_CORES))
    )
    return gather(res)


# revision 34
# speedup vs baseline: 1.2193x; 1.0158x over previous
"""BertSelfAttention (ALiBi-style additive bias) on 8 TRN2 NeuronCores.

Problem: B=4, S=1024, D=1024, H=16 heads (HD=64), fp32.
  qkv = hidden @ Wqkv_w.T + Wqkv_b
  scores = q @ k.T / sqrt(64) + bias ;  probs = softmax(scores) ; out = probs @ v

Sharding: 8 cores = 4 batches x 2 head-groups of 8 "slots". Core c handles
batch c//2 and takes one head from each of 8 head PAIRS (parity c%2), so both
cores of a batch run the identical program on equal work.

Key ideas vs the naive kernel:
  * exp(s + b) = exp(s) * exp(b): the additive bias never touches the
    TensorEngine. The host ships exp(bias) (bf16); the device multiplies it
    into exp(scores) on the (otherwise idle) DVE. This removes 128 identity
    matmuls (~35us of PE time) per core.
  * ALiBi block sparsity: bias = -slope_h * |q - k|, so (128x128) score
    blocks with bias < -T contribute < e^-T relative softmax mass and are
    skipped entirely (no QK matmul, no exp, no bias DMA, no AV matmul).
    Heads are paired sparse-with-sparse across the two cores of a batch so
    the shared SPMD program uses per-slot block radii = max over the pair.
    kernel() verifies the dead blocks against the ACTUAL runtime bias values
    and falls back to a dense variant of the same program if the input is
    not ALiBi-shaped.
  * Scores are computed transposed (scoresT[k, q]) so the AV matmul
    [v | 1].T @ expT also yields the softmax denominator in row 64;
    normalization = fast reciprocal + partition-broadcast + DVE multiply.
  * No max-subtraction in softmax: |scores| <= ~10 in fp32 cannot overflow,
    and large-negative ALiBi bias underflows exp to a clean 0.
  * DMA order: the 8 hidden/weight chunks are triggered first (constants
    after) so the first matmul starts as soon as chunk 0 lands.
"""

import numpy as np

import concourse.bacc as bacc
import concourse.bass as bass
import concourse.mybir as mybir
from concourse.tile import TileContext

B, S, D = 4, 1024, 1024
H = 16
HD = 64  # head dim
N_CORES = 8
HPC = 8  # head-slots per core
OC = 3 * HPC * HD  # 1536 fused-qkv output rows per core
F32 = mybir.dt.float32
BF16 = mybir.dt.bfloat16

KC = S // 128  # 8 key-token chunks of 128
DC = D // 128  # 8 contraction chunks of 128

# --- ALiBi sparsity geometry -------------------------------------------------
# Head h (0-indexed) has slope 2^(-8(h+1)/16). A (kc, qc) block of 128x128
# token pairs is dead when its *smallest* |q - k| distance, 128*|kc-qc| - 127,
# gives bias below -T_SPARSE everywhere in the block.
T_SPARSE = 6.0
T_CHECK = 5.5  # runtime verification margin for dead blocks
_DENSE_MIN_R = 4  # pair radii above 3 round up to fully dense (r=7)


def _alibi_radii(thresh: float) -> list[int]:
    slopes = 2.0 ** (-8.0 * (np.arange(1, H + 1) / H))
    radii = []
    for sl in slopes:
        r = 7
        while r >= 1 and 128 * r - 127 > thresh / sl:
            r -= 1
        radii.append(r)
    return radii


def _make_slots(radii: list[int]):
    order = sorted(range(H), key=lambda h: (radii[h], h))
    pairs = [(order[2 * i], order[2 * i + 1]) for i in range(HPC)]
    slot_r = [max(radii[a], radii[b]) for a, b in pairs]
    # densest slots first: the tail (last slot's serial normalize chain)
    # then belongs to the sparsest head, and early slots line up with the
    # first-finished qkv bands.
    perm = sorted(range(HPC), key=lambda i: (-slot_r[i], i))
    # PSUM accumulation groups must have a uniform footprint (region-varying
    # start/stop within a bank silently drops contributions), so sparse slots
    # use per-query-block accumulation groups. That only pays off for small
    # radii; near-dense slots round up to the plain dense pattern.
    slot_r = [r if r < _DENSE_MIN_R else KC - 1 for r in slot_r]
    return [pairs[i] for i in perm], [slot_r[i] for i in perm]


PAIRS, SLOT_R = _make_slots(_alibi_radii(T_SPARSE))
DENSE_R = [7] * HPC


def _window(r: int, kc: int) -> tuple[int, int]:
    """Alive query-block range [lo, hi] (inclusive) for key-chunk kc."""
    return max(0, kc - r), min(KC - 1, kc + r)


def _half_ranges(w0: int, w1: int):
    """Split column range [w0, w1) at the 512 PSUM-bank boundary."""
    out = []
    for hb in range(2):
        c0, c1 = max(w0, hb * 512), min(w1, (hb + 1) * 512)
        if c0 < c1:
            out.append((hb, c0, c1))
    return out


def build_bass(slot_r=None) -> bass.Bass:
    if slot_r is None:
        slot_r = SLOT_R
    nc = bacc.Bacc()

    hw = nc.declare_dram_parameter("hw", [D, S + OC], BF16, isOutput=False)
    wbv = nc.declare_dram_parameter("wbv", [128, HPC * HD], BF16, isOutput=False)
    wbp = nc.declare_dram_parameter("wbp", [128, 8], F32, isOutput=False)
    ebT = nc.declare_dram_parameter("ebT", [HPC, S, S], BF16, isOutput=False)
    oT = nc.declare_dram_parameter("oT", [HPC * HD, S], F32, isOutput=True)

    with TileContext(nc) as tc:
        with (
            tc.tile_pool(name="const", bufs=1) as constp,
            tc.tile_pool(name="weights", bufs=1) as wp,
            tc.tile_pool(name="qk", bufs=1) as qkp,
            tc.tile_pool(name="vex", bufs=1) as vp,
            tc.tile_pool(name="ebias", bufs=10) as ebp,
            tc.tile_pool(name="exp", bufs=7) as ep,
            # sparse qc-major AV reads et2(kc) until qc = kc + 2r + LAG,
            # so keep enough rotation depth for r=3 slots
            tc.tile_pool(name="exp2", bufs=14) as e2p,
            tc.tile_pool(name="outs", bufs=2) as op_,
            tc.tile_pool(name="ps_mm", bufs=2, space="PSUM") as ps_mm,
            tc.tile_pool(name="ps_sm", bufs=4, space="PSUM") as ps_sm,
        ):
            # --- stage inputs ---------------------------------------------
            # hidden^T | W^T chunks first: the first matmul only waits on
            # chunk 0. Small constants ride behind them on the queue.
            hT_sb = []
            wT_sb = []
            for c in range(DC):
                hwt = wp.tile([128, S + OC], BF16, tag=f"hw{c}", name=f"hw{c}")
                nc.sync.dma_start(out=hwt[:], in_=hw[c * 128 : (c + 1) * 128, :])
                hT_sb.append(hwt[:, 0:S])
                wT_sb.append(hwt[:, S : S + OC])

            wbv_sb = constp.tile([128, HPC, HD], BF16)
            nc.sync.dma_start(
                out=wbv_sb[:].rearrange("p h d -> p (h d)"), in_=wbv[:]
            )
            wbp_sb = constp.tile([128, 8], F32)
            nc.sync.dma_start(out=wbp_sb[:], in_=wbp[:])

            # --- phase 1: fused QKV projection -----------------------------
            # qk_sb[j][p, t]: j in 0..3 -> q rows (pre-scaled by 1/8),
            #                 j in 4..7 -> k rows. Row (j%4)*128+p = oc index.
            qk_sb = [
                qkp.tile([128, S], BF16, tag=f"qk{j}", name=f"qk{j}")
                for j in range(8)
            ]
            # v_sb[t][p, s, 0:64] = v slot s, token t*128+p; [.., 64] = 1.0
            v_sb = [
                vp.tile([128, HPC, HD + 1], BF16, tag=f"vx{t}", name=f"v{t}")
                for t in range(KC)
            ]

            def qk_blk(j):
                ps = ps_mm.tile([128, S], F32, tag="mm", name=f"qkp{j}")

                def mm(c):
                    lw = wT_sb[c][:, j * 128 : (j + 1) * 128]
                    for half in range(2):
                        nc.tensor.matmul(
                            ps[:, half * 512 : (half + 1) * 512],
                            lw,
                            hT_sb[c][:, half * 512 : (half + 1) * 512],
                            start=(c == 0),
                            stop=(c == DC - 1),
                        )

                def fin():
                    # copy to SBUF, adding the per-partition qkv bias and
                    # folding the 1/sqrt(HD) score scale into q rows (DVE)
                    if j < 4:
                        nc.vector.tensor_scalar(
                            qk_sb[j][:], ps[:], wbp_sb[:, j : j + 1], 0.125,
                            op0=mybir.AluOpType.add, op1=mybir.AluOpType.mult,
                        )
                    else:
                        nc.vector.tensor_scalar_add(
                            qk_sb[j][:], ps[:], wbp_sb[:, j : j + 1]
                        )

                return mm, fin

            def v_blk(t):
                ps = ps_sm.tile([128, HPC * HD], F32, tag="sm", name=f"vps{t}")

                def mm(c):
                    nc.tensor.matmul(
                        ps[:],
                        hT_sb[c][:, t * 128 : (t + 1) * 128],
                        wT_sb[c][:, 2 * HPC * HD : 3 * HPC * HD],
                        start=(c == 0),
                        stop=(c == DC - 1),
                    )

                def fin():
                    nc.vector.tensor_tensor(
                        v_sb[t][:, :, 0:HD],
                        ps[:].rearrange("p (h d) -> p h d", h=HPC),
                        wbv_sb[:],
                        op=mybir.AluOpType.add,
                    )
                    nc.scalar.activation(
                        v_sb[t][:, :, HD : HD + 1],
                        v_sb[t][:, :, 0:1],
                        mybir.ActivationFunctionType.Identity,
                        scale=0.0,
                        bias=1.0,
                    )

                return mm, fin

            bands = [
                [qk_blk(0), qk_blk(4), v_blk(0), v_blk(1), v_blk(2)],
                [qk_blk(1), qk_blk(5), v_blk(3), v_blk(4), v_blk(5)],
                [qk_blk(2), qk_blk(6), v_blk(6), v_blk(7)],
                [qk_blk(3), qk_blk(7)],
            ]
            for band in bands:
                for c in range(DC):
                    for mm, _ in band:
                        mm(c)
                for _, fin in band:
                    fin()

            # --- phase 2: attention ----------------------------------------
            # Software-pipelined across (slot, k-chunk) items: the AV matmuls
            # are emitted LAG items late so the in-order PE stream never
            # stalls waiting on an item's exp * exp(bias).
            #
            # Dense slots (r=7) accumulate kc-major with full-half groups.
            # Sparse slots accumulate qc-major: per query-block, one
            # uniform-footprint PSUM group over its alive key-chunks
            # (region-varying start/stop within a group loses contributions).
            # Slots are processed in interleaved PAIRS (a,b): [a.kc0, b.kc0,
            # a.kc1, ...]. Pairing a ScalarE-heavy dense slot with a PE-cheap
            # sparse slot keeps both engines below the pipeline rate, and a
            # pair's two pos tiles exactly fill the 4-buffer PSUM rotation.
            order = sorted(range(HPC), key=lambda s: (-slot_r[s], s))
            pairs2 = [(order[i], order[HPC - 1 - i]) for i in range(HPC // 2)]
            items = []
            for a, b in pairs2:
                for kc in range(KC):
                    items.append((a, kc))
                    items.append((b, kc))
            idx_of = {it: i for i, it in enumerate(items)}
            et2s: dict[tuple, object] = {}
            pos_map: dict[int, list] = {}

            def emit_front(i):
                s, kc = items[i]
                r = slot_r[s]
                lo, hi = _window(r, kc)
                w0, w1 = lo * 128, (hi + 1) * 128
                j, po = s // 2, (s % 2) * 64
                qT = qk_sb[j][po : po + 64, :]  # [64, S] (already /8)
                kT = qk_sb[4 + j][po : po + 64, :]  # [64, S]
                ebt = ebp.tile([128, S], BF16, tag="eb", name=f"eb{i}")
                nc.sync.dma_start(
                    out=ebt[:, 0 : w1 - w0],
                    in_=ebT[s, kc * 128 : (kc + 1) * 128, w0:w1],
                )
                ps = ps_mm.tile([128, S], F32, tag="mm", name=f"s{i}")
                # scoresT[k, q] = k @ q.T over the alive window only
                for _hb, c0, c1 in _half_ranges(w0, w1):
                    nc.tensor.matmul(
                        ps[:, c0:c1],
                        kT[:, kc * 128 : (kc + 1) * 128],
                        qT[:, c0:c1],
                        start=True,
                        stop=True,
                    )
                et = ep.tile([128, S], BF16, tag="et", name=f"et{i}")
                nc.scalar.activation(
                    et[:, w0:w1], ps[:, w0:w1], mybir.ActivationFunctionType.Exp
                )
                # fold in the additive bias: exp(s+b) = exp(s)*exp(b)  (DVE)
                et2 = e2p.tile([128, S], BF16, tag="e2", name=f"e2_{i}")
                nc.vector.tensor_tensor(
                    et2[:, w0:w1],
                    et[:, w0:w1],
                    ebt[:, 0 : w1 - w0],
                    op=mybir.AluOpType.mult,
                )
                et2s[(s, kc)] = et2

            def get_pos(s):
                if s not in pos_map:
                    # [65, 512] 1-bank output tiles: rows 0..63 = outT,
                    # row 64 = sum of exp over the alive band
                    pos_map[s] = [
                        ps_sm.tile([HD + 1, 512], F32, tag="sm", name=f"po{s}_{k}")
                        for k in range(2)
                    ]
                return pos_map[s]

            def emit_back_dense(s, kc):
                pos = get_pos(s)
                et2 = et2s[(s, kc)]
                for hb in range(2):
                    nc.tensor.matmul(
                        pos[hb][:],
                        v_sb[kc][:, s, :],
                        et2[:, hb * 512 : (hb + 1) * 512],
                        start=(kc == 0),
                        stop=(kc == KC - 1),
                    )
                if kc == KC - 1:
                    for kc2 in range(KC):
                        del et2s[(s, kc2)]
                    emit_tail(s, 0)
                    emit_tail(s, 1)

            def emit_back_sparse(s, qc):
                r = slot_r[s]
                pos = get_pos(s)
                hb, c0 = qc // 4, (qc % 4) * 128
                klo, khi = max(0, qc - r), min(KC - 1, qc + r)
                for kc in range(klo, khi + 1):
                    nc.tensor.matmul(
                        pos[hb][:, c0 : c0 + 128],
                        v_sb[kc][:, s, :],
                        et2s[(s, kc)][:, qc * 128 : (qc + 1) * 128],
                        start=(kc == klo),
                        stop=(kc == khi),
                    )
                if qc == 3:
                    # queries 0..511 complete: normalize half 0 early
                    emit_tail(s, 0)
                if qc == KC - 1:
                    for kc in range(KC):
                        del et2s[(s, kc)]
                    emit_tail(s, 1)

            def emit_tail(s, half):
                # normalize: out[d,q] * (1/sum[q]).  1/sum via
                # reciprocal_approx_fast (18 bits; sums are benign), broadcast
                # along partitions on the idle GpSimd, multiply on DVE.
                if True:
                    p = get_pos(s)[half]
                    # the sum row lives at PSUM partition 64; DVE cannot
                    # read partition 64 into partition 0, ScalarE can
                    smf = op_.tile([1, 512], F32, tag="smf")
                    nc.scalar.activation(
                        smf[:], p[HD : HD + 1, :],
                        mybir.ActivationFunctionType.Copy,
                    )
                    rcf = op_.tile([1, 512], F32, tag="rcf")
                    nc.vector.reciprocal_approx_fast(rcf[:], smf[:])
                    rb = op_.tile([HD, 512], F32, tag="rb")
                    nc.gpsimd.partition_broadcast(rb[:], rcf[:])
                    ot = op_.tile([HD, 512], F32, tag="ot")
                    nc.vector.tensor_tensor(
                        ot[:], p[0:HD, :], rb[:], op=mybir.AluOpType.mult
                    )
                    nc.sync.dma_start(
                        out=oT[
                            s * HD : (s + 1) * HD, half * 512 : (half + 1) * 512
                        ],
                        in_=ot[:],
                    )

            # backs: (emit-at front index, emit fn). Dense back (s,kc) needs
            # front (s,kc); sparse back (s,qc) needs fronts through qc+r.
            # Sparse items are small (short exp/mult chains), so they ride
            # closer behind their fronts. Normalize tails are deferred a
            # couple of items past their last AV so the ScalarE copy / DVE
            # reciprocal never block the in-order engine queues waiting for
            # the AV accumulation to finish.
            backs = []
            for s in range(HPC):
                r = slot_r[s]
                if r >= KC - 1:
                    for kc in range(KC):
                        backs.append(
                            (
                                idx_of[(s, kc)] + 6,
                                lambda s=s, kc=kc: emit_back_dense(s, kc),
                            )
                        )
                else:
                    for qc in range(KC):
                        backs.append(
                            (
                                idx_of[(s, min(KC - 1, qc + r))] + 5,
                                lambda s=s, qc=qc: emit_back_sparse(s, qc),
                            )
                        )
            backs.sort(key=lambda b: b[0])
            bi = 0
            for i in range(len(items)):
                emit_front(i)
                while bi < len(backs) and backs[bi][0] <= i:
                    backs[bi][1]()
                    bi += 1
            while bi < len(backs):
                backs[bi][1]()
                bi += 1

    # Bacc defers register allocation to its compile() pass, which only runs
    # in finalize(); run_bass_via_pjrt ships the BIR as-is, so finalize here.
    nc.finalize()
    return nc


def shard_inputs(hidden_states, bias, Wqkv_w, Wqkv_b):
    """Slice + lay out the full inputs into 8 per-core input maps."""
    import ml_dtypes

    bf16 = ml_dtypes.bfloat16
    hidden_states = np.asarray(hidden_states, dtype=np.float32)
    bias = np.asarray(bias, dtype=np.float32)
    Wqkv_w = np.asarray(Wqkv_w, dtype=np.float32)
    Wqkv_b = np.asarray(Wqkv_b, dtype=np.float32)

    in_maps = []
    for c in range(N_CORES):
        b, par = c // 2, c % 2
        heads = [PAIRS[s][par] for s in range(HPC)]
        rows = np.concatenate(
            [np.arange(g * D + h * HD, g * D + (h + 1) * HD) for g in range(3) for h in heads]
        )
        wbp2 = np.ascontiguousarray(
            Wqkv_b[rows[0 : 2 * HPC * HD]].reshape(8, 128).T
        ).astype(np.float32)
        wbv2 = np.broadcast_to(
            Wqkv_b[rows[2 * HPC * HD :]].astype(bf16)[None, :], (128, HPC * HD)
        )
        eb = np.exp(bias[b, heads])  # [8, S, S] fp32
        in_maps.append(
            {
                "hw": np.concatenate(
                    [hidden_states[b].T, Wqkv_w[rows].T], axis=1
                ).astype(bf16),
                "wbv": np.ascontiguousarray(wbv2),
                "wbp": wbp2,
                "ebT": np.ascontiguousarray(eb.transpose(0, 2, 1)).astype(bf16),
            }
        )
    return in_maps


def gather(res):
    out = np.empty((B, S, D), dtype=np.float32)
    for c in range(N_CORES):
        b, par = c // 2, c % 2
        for s in range(HPC):
            h = PAIRS[s][par]
            out[b, :, h * HD : (h + 1) * HD] = res.results[c]["oT"][
                s * HD : (s + 1) * HD, :
            ].T
    return out


def sparsity_ok(bias) -> bool:
    """Verify the ACTUAL bias values keep every skipped block below -T_CHECK
    (so its softmax mass is < ~e^-12 of the row total)."""
    if all(r >= KC - 1 for r in SLOT_R):
        return True
    bias = np.asarray(bias, dtype=np.float32)
    bm = bias.reshape(B, H, KC, 128, KC, 128).max(axis=(3, 5))  # [B,H,8,8]
    kc = np.arange(KC)[:, None]
    qc = np.arange(KC)[None, :]
    for s, r in enumerate(SLOT_R):
        dead = np.abs(kc - qc) > r
        if not dead.any():
            continue
        for h in PAIRS[s]:
            if not np.all(bm[:, h][:, dead] < -T_CHECK):
                return False
    return True


_CACHED = {}


def kernel(hidden_states, bias, Wqkv_w, Wqkv_b):
    from concourse.bass_utils import run_bass_kernel_spmd

    slot_r = tuple(SLOT_R if sparsity_ok(bias) else DENSE_R)
    if slot_r not in _CACHED:
        _CACHED[slot_r] = build_bass(list(slot_r))
    in_maps = shard_inputs(hidden_states, bias, Wqkv_w, Wqkv_b)
    res = run_bass_kernel_spmd(
        _CACHED[slot_r], in_maps, core_ids=list(range(N_CORES))
    )
    return gather(res)


# revision 35
# speedup vs baseline: 1.2364x; 1.0140x over previous
"""BertSelfAttention (ALiBi-style additive bias) on 8 TRN2 NeuronCores.

Problem: B=4, S=1024, D=1024, H=16 heads (HD=64), fp32.
  qkv = hidden @ Wqkv_w.T + Wqkv_b
  scores = q @ k.T / sqrt(64) + bias ;  probs = softmax(scores) ; out = probs @ v

Sharding: 8 cores = 4 batches x 2 head-groups of 8 "slots". Core c handles
batch c//2 and takes one head from each of 8 head PAIRS (parity c%2), so both
cores of a batch run the identical program on equal work.

Key ideas vs the naive kernel:
  * exp(s + b) = exp(s) * exp(b): the additive bias never touches the
    TensorEngine. The host ships exp(bias) (bf16); the device multiplies it
    into exp(scores) on the (otherwise idle) DVE. This removes 128 identity
    matmuls (~35us of PE time) per core.
  * ALiBi block sparsity: bias = -slope_h * |q - k|, so (128x128) score
    blocks with bias < -T contribute < e^-T relative softmax mass and are
    skipped entirely (no QK matmul, no exp, no bias DMA, no AV matmul).
    Heads are paired sparse-with-sparse across the two cores of a batch so
    the shared SPMD program uses per-slot block radii = max over the pair.
    kernel() verifies the dead blocks against the ACTUAL runtime bias values
    and falls back to a dense variant of the same program if the input is
    not ALiBi-shaped.
  * Scores are computed transposed (scoresT[k, q]) so the AV matmul
    [v | 1].T @ expT also yields the softmax denominator in row 64;
    normalization = fast reciprocal + partition-broadcast + DVE multiply.
  * No max-subtraction in softmax: |scores| <= ~10 in fp32 cannot overflow,
    and large-negative ALiBi bias underflows exp to a clean 0.
  * DMA order: the 8 hidden/weight chunks are triggered first (constants
    after) so the first matmul starts as soon as chunk 0 lands.
"""

import numpy as np

import concourse.bacc as bacc
import concourse.bass as bass
import concourse.mybir as mybir
from concourse.tile import TileContext

B, S, D = 4, 1024, 1024
H = 16
HD = 64  # head dim
N_CORES = 8
HPC = 8  # head-slots per core
OC = 3 * HPC * HD  # 1536 fused-qkv output rows per core
F32 = mybir.dt.float32
BF16 = mybir.dt.bfloat16

KC = S // 128  # 8 key-token chunks of 128
DC = D // 128  # 8 contraction chunks of 128

# --- ALiBi sparsity geometry -------------------------------------------------
# Head h (0-indexed) has slope 2^(-8(h+1)/16). A (kc, qc) block of 128x128
# token pairs is dead when its *smallest* |q - k| distance, 128*|kc-qc| - 127,
# gives bias below -T_SPARSE everywhere in the block.
T_SPARSE = 6.0
T_CHECK = 5.5  # runtime verification margin for dead blocks
_DENSE_MIN_R = 4  # pair radii above 3 round up to fully dense (r=7)


def _alibi_radii(thresh: float) -> list[int]:
    slopes = 2.0 ** (-8.0 * (np.arange(1, H + 1) / H))
    radii = []
    for sl in slopes:
        r = 7
        while r >= 1 and 128 * r - 127 > thresh / sl:
            r -= 1
        radii.append(r)
    return radii


def _make_slots(radii: list[int]):
    order = sorted(range(H), key=lambda h: (radii[h], h))
    pairs = [(order[2 * i], order[2 * i + 1]) for i in range(HPC)]
    slot_r = [max(radii[a], radii[b]) for a, b in pairs]
    # densest slots first: the tail (last slot's serial normalize chain)
    # then belongs to the sparsest head, and early slots line up with the
    # first-finished qkv bands.
    perm = sorted(range(HPC), key=lambda i: (-slot_r[i], i))
    # PSUM accumulation groups must have a uniform footprint (region-varying
    # start/stop within a bank silently drops contributions), so sparse slots
    # use per-query-block accumulation groups. That only pays off for small
    # radii; near-dense slots round up to the plain dense pattern.
    slot_r = [r if r < _DENSE_MIN_R else KC - 1 for r in slot_r]
    return [pairs[i] for i in perm], [slot_r[i] for i in perm]


PAIRS, SLOT_R = _make_slots(_alibi_radii(T_SPARSE))
DENSE_R = [7] * HPC


def _window(r: int, kc: int) -> tuple[int, int]:
    """Alive query-block range [lo, hi] (inclusive) for key-chunk kc."""
    return max(0, kc - r), min(KC - 1, kc + r)


def _half_ranges(w0: int, w1: int):
    """Split column range [w0, w1) at the 512 PSUM-bank boundary."""
    out = []
    for hb in range(2):
        c0, c1 = max(w0, hb * 512), min(w1, (hb + 1) * 512)
        if c0 < c1:
            out.append((hb, c0, c1))
    return out


def build_bass(slot_r=None) -> bass.Bass:
    if slot_r is None:
        slot_r = SLOT_R
    nc = bacc.Bacc()

    hw = nc.declare_dram_parameter("hw", [D, S + OC], BF16, isOutput=False)
    wbv = nc.declare_dram_parameter("wbv", [128, HPC * HD], BF16, isOutput=False)
    wbp = nc.declare_dram_parameter("wbp", [128, 8], F32, isOutput=False)
    ebT = nc.declare_dram_parameter("ebT", [HPC, S, S], BF16, isOutput=False)
    oT = nc.declare_dram_parameter("oT", [HPC * HD, S], F32, isOutput=True)

    with TileContext(nc) as tc:
        with (
            tc.tile_pool(name="const", bufs=1) as constp,
            tc.tile_pool(name="weights", bufs=1) as wp,
            tc.tile_pool(name="qk", bufs=1) as qkp,
            tc.tile_pool(name="vex", bufs=1) as vp,
            tc.tile_pool(name="ebias", bufs=12) as ebp,
            tc.tile_pool(name="exp", bufs=8) as ep,
            # sparse qc-major AV reads et2(kc) until qc = kc + 2r + LAG,
            # so keep enough rotation depth for r=3 slots
            tc.tile_pool(name="exp2", bufs=16) as e2p,
            tc.tile_pool(name="outs", bufs=4) as op_,
            tc.tile_pool(name="ps_mm", bufs=2, space="PSUM") as ps_mm,
            tc.tile_pool(name="ps_sm", bufs=4, space="PSUM") as ps_sm,
        ):
            # --- stage inputs ---------------------------------------------
            # hidden^T | W^T chunks first: the first matmul only waits on
            # chunk 0. Small constants ride behind them on the queue.
            hT_sb = []
            wT_sb = []
            for c in range(DC):
                hwt = wp.tile([128, S + OC], BF16, tag=f"hw{c}", name=f"hw{c}")
                nc.sync.dma_start(out=hwt[:], in_=hw[c * 128 : (c + 1) * 128, :])
                hT_sb.append(hwt[:, 0:S])
                wT_sb.append(hwt[:, S : S + OC])

            wbv_sb = constp.tile([128, HPC, HD], BF16)
            nc.sync.dma_start(
                out=wbv_sb[:].rearrange("p h d -> p (h d)"), in_=wbv[:]
            )
            wbp_sb = constp.tile([128, 8], F32)
            nc.sync.dma_start(out=wbp_sb[:], in_=wbp[:])

            # --- phase 1: fused QKV projection -----------------------------
            # qk_sb[j][p, t]: j in 0..3 -> q rows (pre-scaled by 1/8),
            #                 j in 4..7 -> k rows. Row (j%4)*128+p = oc index.
            qk_sb = [
                qkp.tile([128, S], BF16, tag=f"qk{j}", name=f"qk{j}")
                for j in range(8)
            ]
            # v_sb[t][p, s, 0:64] = v slot s, token t*128+p; [.., 64] = 1.0
            v_sb = [
                vp.tile([128, HPC, HD + 1], BF16, tag=f"vx{t}", name=f"v{t}")
                for t in range(KC)
            ]

            def qk_blk(j):
                ps = ps_mm.tile([128, S], F32, tag="mm", name=f"qkp{j}")

                def mm(c):
                    lw = wT_sb[c][:, j * 128 : (j + 1) * 128]
                    for half in range(2):
                        nc.tensor.matmul(
                            ps[:, half * 512 : (half + 1) * 512],
                            lw,
                            hT_sb[c][:, half * 512 : (half + 1) * 512],
                            start=(c == 0),
                            stop=(c == DC - 1),
                        )

                def fin():
                    # copy to SBUF, adding the per-partition qkv bias and
                    # folding the 1/sqrt(HD) score scale into q rows (DVE)
                    if j < 4:
                        nc.vector.tensor_scalar(
                            qk_sb[j][:], ps[:], wbp_sb[:, j : j + 1], 0.125,
                            op0=mybir.AluOpType.add, op1=mybir.AluOpType.mult,
                        )
                    else:
                        nc.vector.tensor_scalar_add(
                            qk_sb[j][:], ps[:], wbp_sb[:, j : j + 1]
                        )

                return mm, fin

            def v_blk(t):
                ps = ps_sm.tile([128, HPC * HD], F32, tag="sm", name=f"vps{t}")

                def mm(c):
                    nc.tensor.matmul(
                        ps[:],
                        hT_sb[c][:, t * 128 : (t + 1) * 128],
                        wT_sb[c][:, 2 * HPC * HD : 3 * HPC * HD],
                        start=(c == 0),
                        stop=(c == DC - 1),
                    )

                def fin():
                    nc.vector.tensor_tensor(
                        v_sb[t][:, :, 0:HD],
                        ps[:].rearrange("p (h d) -> p h d", h=HPC),
                        wbv_sb[:],
                        op=mybir.AluOpType.add,
                    )
                    nc.scalar.activation(
                        v_sb[t][:, :, HD : HD + 1],
                        v_sb[t][:, :, 0:1],
                        mybir.ActivationFunctionType.Identity,
                        scale=0.0,
                        bias=1.0,
                    )

                return mm, fin

            bands = [
                [qk_blk(0), qk_blk(4), v_blk(0), v_blk(1), v_blk(2)],
                [qk_blk(1), qk_blk(5), v_blk(3), v_blk(4), v_blk(5)],
                [qk_blk(2), qk_blk(6), v_blk(6), v_blk(7)],
                [qk_blk(3), qk_blk(7)],
            ]
            for band in bands:
                for c in range(DC):
                    for mm, _ in band:
                        mm(c)
                for _, fin in band:
                    fin()

            # --- phase 2: attention ----------------------------------------
            # Software-pipelined across (slot, k-chunk) items: the AV matmuls
            # are emitted LAG items late so the in-order PE stream never
            # stalls waiting on an item's exp * exp(bias).
            #
            # Dense slots (r=7) accumulate kc-major with full-half groups.
            # Sparse slots accumulate qc-major: per query-block, one
            # uniform-footprint PSUM group over its alive key-chunks
            # (region-varying start/stop within a group loses contributions).
            # Slots are processed in interleaved PAIRS (a,b): [a.kc0, b.kc0,
            # a.kc1, ...]. Pairing a ScalarE-heavy dense slot with a PE-cheap
            # sparse slot keeps both engines below the pipeline rate, and a
            # pair's two pos tiles exactly fill the 4-buffer PSUM rotation.
            order = sorted(range(HPC), key=lambda s: (-slot_r[s], s))
            pairs2 = [(order[i], order[HPC - 1 - i]) for i in range(HPC // 2)]
            items = []
            for a, b in pairs2:
                for kc in range(KC):
                    items.append((a, kc))
                    items.append((b, kc))
            idx_of = {it: i for i, it in enumerate(items)}
            et2s: dict[tuple, object] = {}
            pos_map: dict[int, list] = {}

            def emit_front(i):
                s, kc = items[i]
                r = slot_r[s]
                lo, hi = _window(r, kc)
                w0, w1 = lo * 128, (hi + 1) * 128
                j, po = s // 2, (s % 2) * 64
                qT = qk_sb[j][po : po + 64, :]  # [64, S] (already /8)
                kT = qk_sb[4 + j][po : po + 64, :]  # [64, S]
                ebt = ebp.tile([128, S], BF16, tag="eb", name=f"eb{i}")
                nc.sync.dma_start(
                    out=ebt[:, 0 : w1 - w0],
                    in_=ebT[s, kc * 128 : (kc + 1) * 128, w0:w1],
                )
                ps = ps_mm.tile([128, S], F32, tag="mm", name=f"s{i}")
                # scoresT[k, q] = k @ q.T over the alive window only
                for _hb, c0, c1 in _half_ranges(w0, w1):
                    nc.tensor.matmul(
                        ps[:, c0:c1],
                        kT[:, kc * 128 : (kc + 1) * 128],
                        qT[:, c0:c1],
                        start=True,
                        stop=True,
                    )
                et = ep.tile([128, S], BF16, tag="et", name=f"et{i}")
                nc.scalar.activation(
                    et[:, w0:w1], ps[:, w0:w1], mybir.ActivationFunctionType.Exp
                )
                # fold in the additive bias: exp(s+b) = exp(s)*exp(b)  (DVE)
                et2 = e2p.tile([128, S], BF16, tag="e2", name=f"e2_{i}")
                nc.vector.tensor_tensor(
                    et2[:, w0:w1],
                    et[:, w0:w1],
                    ebt[:, 0 : w1 - w0],
                    op=mybir.AluOpType.mult,
                )
                et2s[(s, kc)] = et2

            def get_pos(s):
                if s not in pos_map:
                    # [65, 512] 1-bank output tiles: rows 0..63 = outT,
                    # row 64 = sum of exp over the alive band
                    pos_map[s] = [
                        ps_sm.tile([HD + 1, 512], F32, tag="sm", name=f"po{s}_{k}")
                        for k in range(2)
                    ]
                return pos_map[s]

            def emit_back_dense(s, kc):
                pos = get_pos(s)
                et2 = et2s[(s, kc)]
                for hb in range(2):
                    nc.tensor.matmul(
                        pos[hb][:],
                        v_sb[kc][:, s, :],
                        et2[:, hb * 512 : (hb + 1) * 512],
                        start=(kc == 0),
                        stop=(kc == KC - 1),
                    )
                if kc == KC - 1:
                    for kc2 in range(KC):
                        del et2s[(s, kc2)]
                    emit_tail(s, 0)
                    emit_tail(s, 1)

            def emit_back_sparse(s, qc):
                r = slot_r[s]
                pos = get_pos(s)
                hb, c0 = qc // 4, (qc % 4) * 128
                klo, khi = max(0, qc - r), min(KC - 1, qc + r)
                for kc in range(klo, khi + 1):
                    nc.tensor.matmul(
                        pos[hb][:, c0 : c0 + 128],
                        v_sb[kc][:, s, :],
                        et2s[(s, kc)][:, qc * 128 : (qc + 1) * 128],
                        start=(kc == klo),
                        stop=(kc == khi),
                    )
                if qc == 3:
                    # queries 0..511 complete: normalize half 0 early
                    emit_tail(s, 0)
                if qc == KC - 1:
                    for kc in range(KC):
                        del et2s[(s, kc)]
                    emit_tail(s, 1)

            def emit_tail(s, half):
                # normalize: out[d,q] * (1/sum[q]).  1/sum via
                # reciprocal_approx_fast (18 bits; sums are benign), broadcast
                # along partitions on the idle GpSimd, multiply on DVE.
                if True:
                    p = get_pos(s)[half]
                    # the sum row lives at PSUM partition 64; DVE cannot
                    # read partition 64 into partition 0, ScalarE can
                    smf = op_.tile([1, 512], F32, tag="smf")
                    nc.scalar.activation(
                        smf[:], p[HD : HD + 1, :],
                        mybir.ActivationFunctionType.Copy,
                    )
                    rcf = op_.tile([1, 512], F32, tag="rcf")
                    nc.vector.reciprocal_approx_fast(rcf[:], smf[:])
                    rb = op_.tile([HD, 512], F32, tag="rb")
                    nc.gpsimd.partition_broadcast(rb[:], rcf[:])
                    ot = op_.tile([HD, 512], F32, tag="ot")
                    nc.vector.tensor_tensor(
                        ot[:], p[0:HD, :], rb[:], op=mybir.AluOpType.mult
                    )
                    nc.sync.dma_start(
                        out=oT[
                            s * HD : (s + 1) * HD, half * 512 : (half + 1) * 512
                        ],
                        in_=ot[:],
                    )

            # backs: (emit-at front index, emit fn). Dense back (s,kc) needs
            # front (s,kc); sparse back (s,qc) needs fronts through qc+r.
            # Sparse items are small (short exp/mult chains), so they ride
            # closer behind their fronts. Normalize tails are deferred a
            # couple of items past their last AV so the ScalarE copy / DVE
            # reciprocal never block the in-order engine queues waiting for
            # the AV accumulation to finish.
            backs = []
            for s in range(HPC):
                r = slot_r[s]
                if r >= KC - 1:
                    for kc in range(KC):
                        backs.append(
                            (
                                idx_of[(s, kc)] + 8,
                                lambda s=s, kc=kc: emit_back_dense(s, kc),
                            )
                        )
                else:
                    for qc in range(KC):
                        backs.append(
                            (
                                idx_of[(s, min(KC - 1, qc + r))] + 7,
                                lambda s=s, qc=qc: emit_back_sparse(s, qc),
                            )
                        )
            backs.sort(key=lambda b: b[0])
            bi = 0
            for i in range(len(items)):
                emit_front(i)
                while bi < len(backs) and backs[bi][0] <= i:
                    backs[bi][1]()
                    bi += 1
            while bi < len(backs):
                backs[bi][1]()
                bi += 1

    # Bacc defers register allocation to its compile() pass, which only runs
    # in finalize(); run_bass_via_pjrt ships the BIR as-is, so finalize here.
    nc.finalize()
    return nc


def shard_inputs(hidden_states, bias, Wqkv_w, Wqkv_b):
    """Slice + lay out the full inputs into 8 per-core input maps."""
    import ml_dtypes

    bf16 = ml_dtypes.bfloat16
    hidden_states = np.asarray(hidden_states, dtype=np.float32)
    bias = np.asarray(bias, dtype=np.float32)
    Wqkv_w = np.asarray(Wqkv_w, dtype=np.float32)
    Wqkv_b = np.asarray(Wqkv_b, dtype=np.float32)

    in_maps = []
    for c in range(N_CORES):
        b, par = c // 2, c % 2
        heads = [PAIRS[s][par] for s in range(HPC)]
        rows = np.concatenate(
            [np.arange(g * D + h * HD, g * D + (h + 1) * HD) for g in range(3) for h in heads]
        )
        wbp2 = np.ascontiguousarray(
            Wqkv_b[rows[0 : 2 * HPC * HD]].reshape(8, 128).T
        ).astype(np.float32)
        wbv2 = np.broadcast_to(
            Wqkv_b[rows[2 * HPC * HD :]].astype(bf16)[None, :], (128, HPC * HD)
        )
        eb = np.exp(bias[b, heads])  # [8, S, S] fp32
        in_maps.append(
            {
                "hw": np.concatenate(
                    [hidden_states[b].T, Wqkv_w[rows].T], axis=1
                ).astype(bf16),
                "wbv": np.ascontiguousarray(wbv2),
                "wbp": wbp2,
                "ebT": np.ascontiguousarray(eb.transpose(0, 2, 1)).astype(bf16),
            }
        )
    return in_maps


def gather(res):
    out = np.empty((B, S, D), dtype=np.float32)
    for c in range(N_CORES):
        b, par = c // 2, c % 2
        for s in range(HPC):
            h = PAIRS[s][par]
            out[b, :, h * HD : (h + 1) * HD] = res.results[c]["oT"][
                s * HD : (s + 1) * HD, :
            ].T
    return out


def sparsity_ok(bias) -> bool:
    """Verify the ACTUAL bias values keep every skipped block below -T_CHECK
    (so its softmax mass is < ~e^-12 of the row total)."""
    if all(r >= KC - 1 for r in SLOT_R):
        return True
    bias = np.asarray(bias, dtype=np.float32)
    bm = bias.reshape(B, H, KC, 128, KC, 128).max(axis=(3, 5))  # [B,H,8,8]
    kc = np.arange(KC)[:, None]
    qc = np.arange(KC)[None, :]
    for s, r in enumerate(SLOT_R):
        dead = np.abs(kc - qc) > r
        if not dead.any():
            continue
        for h in PAIRS[s]:
            if not np.all(bm[:, h][:, dead] < -T_CHECK):
                return False
    return True


_CACHED = {}


def kernel(hidden_states, bias, Wqkv_w, Wqkv_b):
    from concourse.bass_utils import run_bass_kernel_spmd

    slot_r = tuple(SLOT_R if sparsity_ok(bias) else DENSE_R)
    if slot_r not in _CACHED:
        _CACHED[slot_r] = build_bass(list(slot_r))
    in_maps = shard_inputs(hidden_states, bias, Wqkv_w, Wqkv_b)
    res = run_bass_kernel_spmd(
        _CACHED[slot_r], in_maps, core_ids=list(range(N_CORES))
    )
    return gather(res)


# revision 36
# speedup vs baseline: 1.2410x; 1.0037x over previous
"""BertSelfAttention (ALiBi-style additive bias) on 8 TRN2 NeuronCores.

Problem: B=4, S=1024, D=1024, H=16 heads (HD=64), fp32.
  qkv = hidden @ Wqkv_w.T + Wqkv_b
  scores = q @ k.T / sqrt(64) + bias ;  probs = softmax(scores) ; out = probs @ v

Sharding: 8 cores = 4 batches x 2 head-groups of 8 "slots". Core c handles
batch c//2 and takes one head from each of 8 head PAIRS (parity c%2), so both
cores of a batch run the identical program on equal work.

Key ideas vs the naive kernel:
  * exp(s + b) = exp(s) * exp(b): the additive bias never touches the
    TensorEngine. The host ships exp(bias) (bf16); the device multiplies it
    into exp(scores) on the (otherwise idle) DVE. This removes 128 identity
    matmuls (~35us of PE time) per core.
  * ALiBi block sparsity: bias = -slope_h * |q - k|, so (128x128) score
    blocks with bias < -T contribute < e^-T relative softmax mass and are
    skipped entirely (no QK matmul, no exp, no bias DMA, no AV matmul).
    Heads are paired sparse-with-sparse across the two cores of a batch so
    the shared SPMD program uses per-slot block radii = max over the pair.
    kernel() verifies the dead blocks against the ACTUAL runtime bias values
    and falls back to a dense variant of the same program if the input is
    not ALiBi-shaped.
  * Scores are computed transposed (scoresT[k, q]) so the AV matmul
    [v | 1].T @ expT also yields the softmax denominator in row 64;
    normalization = fast reciprocal + partition-broadcast + DVE multiply.
  * No max-subtraction in softmax: |scores| <= ~10 in fp32 cannot overflow,
    and large-negative ALiBi bias underflows exp to a clean 0.
  * DMA order: the 8 hidden/weight chunks are triggered first (constants
    after) so the first matmul starts as soon as chunk 0 lands.
"""

import numpy as np

import concourse.bacc as bacc
import concourse.bass as bass
import concourse.mybir as mybir
from concourse.tile import TileContext

B, S, D = 4, 1024, 1024
H = 16
HD = 64  # head dim
N_CORES = 8
HPC = 8  # head-slots per core
OC = 3 * HPC * HD  # 1536 fused-qkv output rows per core
F32 = mybir.dt.float32
BF16 = mybir.dt.bfloat16

KC = S // 128  # 8 key-token chunks of 128
DC = D // 128  # 8 contraction chunks of 128

# --- ALiBi sparsity geometry -------------------------------------------------
# Head h (0-indexed) has slope 2^(-8(h+1)/16). A (kc, qc) block of 128x128
# token pairs is dead when its *smallest* |q - k| distance, 128*|kc-qc| - 127,
# gives bias below -T_SPARSE everywhere in the block.
T_SPARSE = 6.0
T_CHECK = 5.5  # runtime verification margin for dead blocks
_DENSE_MIN_R = 4  # pair radii above 3 round up to fully dense (r=7)


def _alibi_radii(thresh: float) -> list[int]:
    slopes = 2.0 ** (-8.0 * (np.arange(1, H + 1) / H))
    radii = []
    for sl in slopes:
        r = 7
        while r >= 1 and 128 * r - 127 > thresh / sl:
            r -= 1
        radii.append(r)
    return radii


def _make_slots(radii: list[int]):
    order = sorted(range(H), key=lambda h: (radii[h], h))
    pairs = [(order[2 * i], order[2 * i + 1]) for i in range(HPC)]
    slot_r = [max(radii[a], radii[b]) for a, b in pairs]
    # densest slots first: the tail (last slot's serial normalize chain)
    # then belongs to the sparsest head, and early slots line up with the
    # first-finished qkv bands.
    perm = sorted(range(HPC), key=lambda i: (-slot_r[i], i))
    # PSUM accumulation groups must have a uniform footprint (region-varying
    # start/stop within a bank silently drops contributions), so sparse slots
    # use per-query-block accumulation groups. That only pays off for small
    # radii; near-dense slots round up to the plain dense pattern.
    slot_r = [r if r < _DENSE_MIN_R else KC - 1 for r in slot_r]
    return [pairs[i] for i in perm], [slot_r[i] for i in perm]


PAIRS, SLOT_R = _make_slots(_alibi_radii(T_SPARSE))
DENSE_R = [7] * HPC


def _window(r: int, kc: int) -> tuple[int, int]:
    """Alive query-block range [lo, hi] (inclusive) for key-chunk kc."""
    return max(0, kc - r), min(KC - 1, kc + r)


def _half_ranges(w0: int, w1: int):
    """Split column range [w0, w1) at the 512 PSUM-bank boundary."""
    out = []
    for hb in range(2):
        c0, c1 = max(w0, hb * 512), min(w1, (hb + 1) * 512)
        if c0 < c1:
            out.append((hb, c0, c1))
    return out


def build_bass(slot_r=None) -> bass.Bass:
    if slot_r is None:
        slot_r = SLOT_R
    nc = bacc.Bacc()

    hw = nc.declare_dram_parameter("hw", [D, S + OC], BF16, isOutput=False)
    wbv = nc.declare_dram_parameter("wbv", [128, HPC * HD], BF16, isOutput=False)
    wbp = nc.declare_dram_parameter("wbp", [128, 8], F32, isOutput=False)
    ebT = nc.declare_dram_parameter("ebT", [HPC, S, S], BF16, isOutput=False)
    oT = nc.declare_dram_parameter("oT", [HPC * HD, S], F32, isOutput=True)

    with TileContext(nc) as tc:
        with (
            tc.tile_pool(name="const", bufs=1) as constp,
            tc.tile_pool(name="weights", bufs=1) as wp,
            tc.tile_pool(name="qk", bufs=1) as qkp,
            tc.tile_pool(name="vex", bufs=1) as vp,
            tc.tile_pool(name="ebias", bufs=12) as ebp,
            tc.tile_pool(name="exp", bufs=8) as ep,
            # sparse qc-major AV reads et2(kc) until qc = kc + 2r + LAG,
            # so keep enough rotation depth for r=3 slots
            tc.tile_pool(name="exp2", bufs=16) as e2p,
            tc.tile_pool(name="outs", bufs=4) as op_,
            tc.tile_pool(name="ps_mm", bufs=2, space="PSUM") as ps_mm,
            tc.tile_pool(name="ps_sm", bufs=4, space="PSUM") as ps_sm,
        ):
            # --- stage inputs ---------------------------------------------
            # hidden^T | W^T chunks first: the first matmul only waits on
            # chunk 0. Small constants ride behind them on the queue.
            hT_sb = []
            wT_sb = []
            for c in range(DC):
                hwt = wp.tile([128, S + OC], BF16, tag=f"hw{c}", name=f"hw{c}")
                nc.sync.dma_start(out=hwt[:], in_=hw[c * 128 : (c + 1) * 128, :])
                hT_sb.append(hwt[:, 0:S])
                wT_sb.append(hwt[:, S : S + OC])

            wbv_sb = constp.tile([128, HPC, HD], BF16)
            nc.sync.dma_start(
                out=wbv_sb[:].rearrange("p h d -> p (h d)"), in_=wbv[:]
            )
            wbp_sb = constp.tile([128, 8], F32)
            nc.sync.dma_start(out=wbp_sb[:], in_=wbp[:])

            # --- phase 1: fused QKV projection -----------------------------
            # qk_sb[j][p, t]: j in 0..3 -> q rows (pre-scaled by 1/8),
            #                 j in 4..7 -> k rows. Row (j%4)*128+p = oc index.
            qk_sb = [
                qkp.tile([128, S], BF16, tag=f"qk{j}", name=f"qk{j}")
                for j in range(8)
            ]
            # v_sb[t][p, s, 0:64] = v slot s, token t*128+p; [.., 64] = 1.0
            v_sb = [
                vp.tile([128, HPC, HD + 1], BF16, tag=f"vx{t}", name=f"v{t}")
                for t in range(KC)
            ]

            def qk_blk(j):
                ps = ps_mm.tile([128, S], F32, tag="mm", name=f"qkp{j}")

                def mm(c):
                    lw = wT_sb[c][:, j * 128 : (j + 1) * 128]
                    for half in range(2):
                        nc.tensor.matmul(
                            ps[:, half * 512 : (half + 1) * 512],
                            lw,
                            hT_sb[c][:, half * 512 : (half + 1) * 512],
                            start=(c == 0),
                            stop=(c == DC - 1),
                        )

                def fin():
                    # copy to SBUF, adding the per-partition qkv bias and
                    # folding the 1/sqrt(HD) score scale into q rows (DVE)
                    if j < 4:
                        nc.vector.tensor_scalar(
                            qk_sb[j][:], ps[:], wbp_sb[:, j : j + 1], 0.125,
                            op0=mybir.AluOpType.add, op1=mybir.AluOpType.mult,
                        )
                    else:
                        nc.vector.tensor_scalar_add(
                            qk_sb[j][:], ps[:], wbp_sb[:, j : j + 1]
                        )

                return mm, fin

            def v_blk(t):
                ps = ps_sm.tile([128, HPC * HD], F32, tag="sm", name=f"vps{t}")

                def mm(c):
                    nc.tensor.matmul(
                        ps[:],
                        hT_sb[c][:, t * 128 : (t + 1) * 128],
                        wT_sb[c][:, 2 * HPC * HD : 3 * HPC * HD],
                        start=(c == 0),
                        stop=(c == DC - 1),
                    )

                def fin():
                    nc.vector.tensor_tensor(
                        v_sb[t][:, :, 0:HD],
                        ps[:].rearrange("p (h d) -> p h d", h=HPC),
                        wbv_sb[:],
                        op=mybir.AluOpType.add,
                    )
                    nc.scalar.activation(
                        v_sb[t][:, :, HD : HD + 1],
                        v_sb[t][:, :, 0:1],
                        mybir.ActivationFunctionType.Identity,
                        scale=0.0,
                        bias=1.0,
                    )

                return mm, fin

            bands = [
                [qk_blk(0), qk_blk(4), v_blk(0), v_blk(1), v_blk(2)],
                [qk_blk(1), qk_blk(5), v_blk(3), v_blk(4), v_blk(5)],
                [qk_blk(2), qk_blk(6), v_blk(6), v_blk(7)],
                [qk_blk(3), qk_blk(7)],
            ]
            for band in bands:
                for c in range(DC):
                    for mm, _ in band:
                        mm(c)
                for _, fin in band:
                    fin()

            # --- phase 2: attention ----------------------------------------
            # Software-pipelined across (slot, k-chunk) items: the AV matmuls
            # are emitted LAG items late so the in-order PE stream never
            # stalls waiting on an item's exp * exp(bias).
            #
            # Dense slots (r=7) accumulate kc-major with full-half groups.
            # Sparse slots accumulate qc-major: per query-block, one
            # uniform-footprint PSUM group over its alive key-chunks
            # (region-varying start/stop within a group loses contributions).
            # Slots are processed in interleaved PAIRS (a,b): [a.kc0, b.kc0,
            # a.kc1, ...]. Pairing a ScalarE-heavy dense slot with a PE-cheap
            # sparse slot keeps both engines below the pipeline rate, and a
            # pair's two pos tiles exactly fill the 4-buffer PSUM rotation.
            order = sorted(range(HPC), key=lambda s: (-slot_r[s], s))
            pairs2 = [(order[i], order[HPC - 1 - i]) for i in range(HPC // 2)]
            items = []
            for a, b in pairs2:
                for kc in range(KC):
                    items.append((a, kc))
                    items.append((b, kc))
            idx_of = {it: i for i, it in enumerate(items)}
            et2s: dict[tuple, object] = {}
            pos_map: dict[int, list] = {}

            def emit_front(i):
                s, kc = items[i]
                r = slot_r[s]
                lo, hi = _window(r, kc)
                w0, w1 = lo * 128, (hi + 1) * 128
                j, po = s // 2, (s % 2) * 64
                qT = qk_sb[j][po : po + 64, :]  # [64, S] (already /8)
                kT = qk_sb[4 + j][po : po + 64, :]  # [64, S]
                ebt = ebp.tile([128, S], BF16, tag="eb", name=f"eb{i}")
                nc.sync.dma_start(
                    out=ebt[:, 0 : w1 - w0],
                    in_=ebT[s, kc * 128 : (kc + 1) * 128, w0:w1],
                )
                ps = ps_mm.tile([128, S], F32, tag="mm", name=f"s{i}")
                # scoresT[k, q] = k @ q.T over the alive window only
                for _hb, c0, c1 in _half_ranges(w0, w1):
                    nc.tensor.matmul(
                        ps[:, c0:c1],
                        kT[:, kc * 128 : (kc + 1) * 128],
                        qT[:, c0:c1],
                        start=True,
                        stop=True,
                    )
                et = ep.tile([128, S], BF16, tag="et", name=f"et{i}")
                nc.scalar.activation(
                    et[:, w0:w1], ps[:, w0:w1], mybir.ActivationFunctionType.Exp
                )
                # fold in the additive bias: exp(s+b) = exp(s)*exp(b)  (DVE)
                et2 = e2p.tile([128, S], BF16, tag="e2", name=f"e2_{i}")
                nc.vector.tensor_tensor(
                    et2[:, w0:w1],
                    et[:, w0:w1],
                    ebt[:, 0 : w1 - w0],
                    op=mybir.AluOpType.mult,
                )
                et2s[(s, kc)] = et2

            def get_pos(s):
                if s not in pos_map:
                    # [65, 512] 1-bank output tiles: rows 0..63 = outT,
                    # row 64 = sum of exp over the alive band
                    pos_map[s] = [
                        ps_sm.tile([HD + 1, 512], F32, tag="sm", name=f"po{s}_{k}")
                        for k in range(2)
                    ]
                return pos_map[s]

            def emit_back_dense(s, kc):
                pos = get_pos(s)
                et2 = et2s[(s, kc)]
                for hb in range(2):
                    nc.tensor.matmul(
                        pos[hb][:],
                        v_sb[kc][:, s, :],
                        et2[:, hb * 512 : (hb + 1) * 512],
                        start=(kc == 0),
                        stop=(kc == KC - 1),
                    )
                if kc == KC - 1:
                    for kc2 in range(KC):
                        del et2s[(s, kc2)]
                    emit_tail(s, 0)
                    emit_tail(s, 1)

            def emit_back_sparse(s, qc):
                r = slot_r[s]
                pos = get_pos(s)
                hb, c0 = qc // 4, (qc % 4) * 128
                klo, khi = max(0, qc - r), min(KC - 1, qc + r)
                for kc in range(klo, khi + 1):
                    nc.tensor.matmul(
                        pos[hb][:, c0 : c0 + 128],
                        v_sb[kc][:, s, :],
                        et2s[(s, kc)][:, qc * 128 : (qc + 1) * 128],
                        start=(kc == klo),
                        stop=(kc == khi),
                    )
                if qc == 3:
                    # queries 0..511 complete: normalize half 0 early
                    emit_tail(s, 0)
                if qc == KC - 1:
                    for kc in range(KC):
                        del et2s[(s, kc)]
                    emit_tail(s, 1)

            def emit_tail(s, half):
                # normalize: out[d,q] * (1/sum[q]).  1/sum via
                # reciprocal_approx_fast (18 bits; sums are benign), broadcast
                # along partitions on the idle GpSimd, multiply on DVE.
                if True:
                    p = get_pos(s)[half]
                    # the sum row lives at PSUM partition 64; DVE cannot
                    # read partition 64 into partition 0, ScalarE can
                    smf = op_.tile([1, 512], F32, tag="smf")
                    nc.scalar.activation(
                        smf[:], p[HD : HD + 1, :],
                        mybir.ActivationFunctionType.Copy,
                    )
                    rcf = op_.tile([1, 512], F32, tag="rcf")
                    nc.vector.reciprocal_approx_fast(rcf[:], smf[:])
                    rb = op_.tile([HD, 512], F32, tag="rb")
                    nc.gpsimd.partition_broadcast(rb[:], rcf[:])
                    ot = op_.tile([HD, 512], F32, tag="ot")
                    nc.vector.tensor_tensor(
                        ot[:], p[0:HD, :], rb[:], op=mybir.AluOpType.mult
                    )
                    nc.sync.dma_start(
                        out=oT[
                            s * HD : (s + 1) * HD, half * 512 : (half + 1) * 512
                        ],
                        in_=ot[:],
                    )

            # backs: (emit-at front index, emit fn). Dense back (s,kc) needs
            # front (s,kc); sparse back (s,qc) needs fronts through qc+r.
            # Sparse items are small (short exp/mult chains), so they ride
            # closer behind their fronts. Normalize tails are deferred a
            # couple of items past their last AV so the ScalarE copy / DVE
            # reciprocal never block the in-order engine queues waiting for
            # the AV accumulation to finish.
            backs = []
            last_pair = set(pairs2[-1])
            for s in range(HPC):
                r = slot_r[s]
                # deep lag overlaps a pair's drain with the next pair's
                # fronts; the final pair has nothing to overlap with, so it
                # rides close behind its fronts instead
                dl, sl = (4, 3) if s in last_pair else (8, 7)
                if r >= KC - 1:
                    for kc in range(KC):
                        backs.append(
                            (
                                idx_of[(s, kc)] + dl,
                                lambda s=s, kc=kc: emit_back_dense(s, kc),
                            )
                        )
                else:
                    for qc in range(KC):
                        backs.append(
                            (
                                idx_of[(s, min(KC - 1, qc + r))] + sl,
                                lambda s=s, qc=qc: emit_back_sparse(s, qc),
                            )
                        )
            backs.sort(key=lambda b: b[0])
            bi = 0
            for i in range(len(items)):
                emit_front(i)
                while bi < len(backs) and backs[bi][0] <= i:
                    backs[bi][1]()
                    bi += 1
            while bi < len(backs):
                backs[bi][1]()
                bi += 1

    # Bacc defers register allocation to its compile() pass, which only runs
    # in finalize(); run_bass_via_pjrt ships the BIR as-is, so finalize here.
    nc.finalize()
    return nc


def shard_inputs(hidden_states, bias, Wqkv_w, Wqkv_b):
    """Slice + lay out the full inputs into 8 per-core input maps."""
    import ml_dtypes

    bf16 = ml_dtypes.bfloat16
    hidden_states = np.asarray(hidden_states, dtype=np.float32)
    bias = np.asarray(bias, dtype=np.float32)
    Wqkv_w = np.asarray(Wqkv_w, dtype=np.float32)
    Wqkv_b = np.asarray(Wqkv_b, dtype=np.float32)

    in_maps = []
    for c in range(N_CORES):
        b, par = c // 2, c % 2
        heads = [PAIRS[s][par] for s in range(HPC)]
        rows = np.concatenate(
            [np.arange(g * D + h * HD, g * D + (h + 1) * HD) for g in range(3) for h in heads]
        )
        wbp2 = np.ascontiguousarray(
            Wqkv_b[rows[0 : 2 * HPC * HD]].reshape(8, 128).T
        ).astype(np.float32)
        wbv2 = np.broadcast_to(
            Wqkv_b[rows[2 * HPC * HD :]].astype(bf16)[None, :], (128, HPC * HD)
        )
        eb = np.exp(bias[b, heads])  # [8, S, S] fp32
        in_maps.append(
            {
                "hw": np.concatenate(
                    [hidden_states[b].T, Wqkv_w[rows].T], axis=1
                ).astype(bf16),
                "wbv": np.ascontiguousarray(wbv2),
                "wbp": wbp2,
                "ebT": np.ascontiguousarray(eb.transpose(0, 2, 1)).astype(bf16),
            }
        )
    return in_maps


def gather(res):
    out = np.empty((B, S, D), dtype=np.float32)
    for c in range(N_CORES):
        b, par = c // 2, c % 2
        for s in range(HPC):
            h = PAIRS[s][par]
            out[b, :, h * HD : (h + 1) * HD] = res.results[c]["oT"][
                s * HD : (s + 1) * HD, :
            ].T
    return out


def sparsity_ok(bias) -> bool:
    """Verify the ACTUAL bias values keep every skipped block below -T_CHECK
    (so its softmax mass is < ~e^-12 of the row total)."""
    if all(r >= KC - 1 for r in SLOT_R):
        return True
    bias = np.asarray(bias, dtype=np.float32)
    bm = bias.reshape(B, H, KC, 128, KC, 128).max(axis=(3, 5))  # [B,H,8,8]
    kc = np.arange(KC)[:, None]
    qc = np.arange(KC)[None, :]
    for s, r in enumerate(SLOT_R):
        dead = np.abs(kc - qc) > r
        if not dead.any():
            continue
        for h in PAIRS[s]:
            if not np.all(bm[:, h][:, dead] < -T_CHECK):
                return False
    return True


_CACHED = {}


def kernel(hidden_states, bias, Wqkv_w, Wqkv_b):
    from concourse.bass_utils import run_bass_kernel_spmd

    slot_r = tuple(SLOT_R if sparsity_ok(bias) else DENSE_R)
    if slot_r not in _CACHED:
        _CACHED[slot_r] = build_bass(list(slot_r))
    in_maps = shard_inputs(hidden_states, bias, Wqkv_w, Wqkv_b)
    res = run_bass_kernel_spmd(
        _CACHED[slot_r], in_maps, core_ids=list(range(N_CORES))
    )
    return gather(res)


# revision 37
# speedup vs baseline: 1.2517x; 1.0087x over previous
"""BertSelfAttention (ALiBi-style additive bias) on 8 TRN2 NeuronCores.

Problem: B=4, S=1024, D=1024, H=16 heads (HD=64), fp32.
  qkv = hidden @ Wqkv_w.T + Wqkv_b
  scores = q @ k.T / sqrt(64) + bias ;  probs = softmax(scores) ; out = probs @ v

Sharding: 8 cores = 4 batches x 2 head-groups of 8 "slots". Core c handles
batch c//2 and takes one head from each of 8 head PAIRS (parity c%2), so both
cores of a batch run the identical program on equal work.

Key ideas vs the naive kernel:
  * exp(s + b) = exp(s) * exp(b): the additive bias never touches the
    TensorEngine. The host ships exp(bias) (bf16); the device multiplies it
    into exp(scores) on the (otherwise idle) DVE. This removes 128 identity
    matmuls (~35us of PE time) per core.
  * ALiBi block sparsity: bias = -slope_h * |q - k|, so (128x128) score
    blocks with bias < -T contribute < e^-T relative softmax mass and are
    skipped entirely (no QK matmul, no exp, no bias DMA, no AV matmul).
    Heads are paired sparse-with-sparse across the two cores of a batch so
    the shared SPMD program uses per-slot block radii = max over the pair.
    kernel() verifies the dead blocks against the ACTUAL runtime bias values
    and falls back to a dense variant of the same program if the input is
    not ALiBi-shaped.
  * Scores are computed transposed (scoresT[k, q]) so the AV matmul
    [v | 1].T @ expT also yields the softmax denominator in row 64;
    normalization = fast reciprocal + partition-broadcast + DVE multiply.
  * No max-subtraction in softmax: |scores| <= ~10 in fp32 cannot overflow,
    and large-negative ALiBi bias underflows exp to a clean 0.
  * DMA order: the 8 hidden/weight chunks are triggered first (constants
    after) so the first matmul starts as soon as chunk 0 lands.
"""

import numpy as np

import concourse.bacc as bacc
import concourse.bass as bass
import concourse.mybir as mybir
from concourse.tile import TileContext

B, S, D = 4, 1024, 1024
H = 16
HD = 64  # head dim
N_CORES = 8
HPC = 8  # head-slots per core
OC = 3 * HPC * HD  # 1536 fused-qkv output rows per core
F32 = mybir.dt.float32
BF16 = mybir.dt.bfloat16

KC = S // 128  # 8 key-token chunks of 128
DC = D // 128  # 8 contraction chunks of 128

# --- ALiBi sparsity geometry -------------------------------------------------
# Head h (0-indexed) has slope 2^(-8(h+1)/16). A (kc, qc) block of 128x128
# token pairs is dead when its *smallest* |q - k| distance, 128*|kc-qc| - 127,
# gives bias below -T_SPARSE everywhere in the block.
T_SPARSE = 6.0
T_CHECK = 5.5  # runtime verification margin for dead blocks
_DENSE_MIN_R = 4  # pair radii above 3 round up to fully dense (r=7)


def _alibi_radii(thresh: float) -> list[int]:
    slopes = 2.0 ** (-8.0 * (np.arange(1, H + 1) / H))
    radii = []
    for sl in slopes:
        r = 7
        while r >= 1 and 128 * r - 127 > thresh / sl:
            r -= 1
        radii.append(r)
    return radii


def _make_slots(radii: list[int]):
    order = sorted(range(H), key=lambda h: (radii[h], h))
    pairs = [(order[2 * i], order[2 * i + 1]) for i in range(HPC)]
    slot_r = [max(radii[a], radii[b]) for a, b in pairs]
    # densest slots first: the tail (last slot's serial normalize chain)
    # then belongs to the sparsest head, and early slots line up with the
    # first-finished qkv bands.
    perm = sorted(range(HPC), key=lambda i: (-slot_r[i], i))
    # PSUM accumulation groups must have a uniform footprint (region-varying
    # start/stop within a bank silently drops contributions), so sparse slots
    # use per-query-block accumulation groups. That only pays off for small
    # radii; near-dense slots round up to the plain dense pattern.
    slot_r = [r if r < _DENSE_MIN_R else KC - 1 for r in slot_r]
    return [pairs[i] for i in perm], [slot_r[i] for i in perm]


PAIRS, SLOT_R = _make_slots(_alibi_radii(T_SPARSE))
DENSE_R = [7] * HPC


def _window(r: int, kc: int) -> tuple[int, int]:
    """Alive query-block range [lo, hi] (inclusive) for key-chunk kc."""
    return max(0, kc - r), min(KC - 1, kc + r)


def _half_ranges(w0: int, w1: int):
    """Split column range [w0, w1) at the 512 PSUM-bank boundary."""
    out = []
    for hb in range(2):
        c0, c1 = max(w0, hb * 512), min(w1, (hb + 1) * 512)
        if c0 < c1:
            out.append((hb, c0, c1))
    return out


def build_bass(slot_r=None) -> bass.Bass:
    if slot_r is None:
        slot_r = SLOT_R
    nc = bacc.Bacc()

    hw = nc.declare_dram_parameter("hw", [D, S + OC], BF16, isOutput=False)
    wbv = nc.declare_dram_parameter("wbv", [128, HPC * HD], BF16, isOutput=False)
    wbp = nc.declare_dram_parameter("wbp", [128, 8], F32, isOutput=False)
    ebT = nc.declare_dram_parameter("ebT", [HPC, S, S], BF16, isOutput=False)
    oT = nc.declare_dram_parameter("oT", [HPC * HD, S], F32, isOutput=True)

    with TileContext(nc) as tc:
        with (
            tc.tile_pool(name="const", bufs=1) as constp,
            tc.tile_pool(name="weights", bufs=1) as wp,
            tc.tile_pool(name="qk", bufs=1) as qkp,
            tc.tile_pool(name="vex", bufs=1) as vp,
            tc.tile_pool(name="ebias", bufs=12) as ebp,
            tc.tile_pool(name="exp", bufs=9) as ep,
            # sparse qc-major AV reads et2(kc) until qc = kc + 2r + LAG,
            # so keep enough rotation depth for r=3 slots
            tc.tile_pool(name="exp2", bufs=18) as e2p,
            tc.tile_pool(name="outs", bufs=4) as op_,
            tc.tile_pool(name="ps_mm", bufs=2, space="PSUM") as ps_mm,
            tc.tile_pool(name="ps_sm", bufs=4, space="PSUM") as ps_sm,
        ):
            # --- stage inputs ---------------------------------------------
            # hidden^T | W^T chunks first: the first matmul only waits on
            # chunk 0. Small constants ride behind them on the queue.
            hT_sb = []
            wT_sb = []
            for c in range(DC):
                hwt = wp.tile([128, S + OC], BF16, tag=f"hw{c}", name=f"hw{c}")
                nc.sync.dma_start(out=hwt[:], in_=hw[c * 128 : (c + 1) * 128, :])
                hT_sb.append(hwt[:, 0:S])
                wT_sb.append(hwt[:, S : S + OC])

            wbv_sb = constp.tile([128, HPC, HD], BF16)
            nc.sync.dma_start(
                out=wbv_sb[:].rearrange("p h d -> p (h d)"), in_=wbv[:]
            )
            wbp_sb = constp.tile([128, 8], F32)
            nc.sync.dma_start(out=wbp_sb[:], in_=wbp[:])

            # --- phase 1: fused QKV projection -----------------------------
            # qk_sb[j][p, t]: j in 0..3 -> q rows (pre-scaled by 1/8),
            #                 j in 4..7 -> k rows. Row (j%4)*128+p = oc index.
            qk_sb = [
                qkp.tile([128, S], BF16, tag=f"qk{j}", name=f"qk{j}")
                for j in range(8)
            ]
            # v_sb[t][p, s, 0:64] = v slot s, token t*128+p; [.., 64] = 1.0
            v_sb = [
                vp.tile([128, HPC, HD + 1], BF16, tag=f"vx{t}", name=f"v{t}")
                for t in range(KC)
            ]

            def qk_blk(j):
                ps = ps_mm.tile([128, S], F32, tag="mm", name=f"qkp{j}")

                def mm(c):
                    lw = wT_sb[c][:, j * 128 : (j + 1) * 128]
                    for half in range(2):
                        nc.tensor.matmul(
                            ps[:, half * 512 : (half + 1) * 512],
                            lw,
                            hT_sb[c][:, half * 512 : (half + 1) * 512],
                            start=(c == 0),
                            stop=(c == DC - 1),
                        )

                def fin():
                    # copy to SBUF, adding the per-partition qkv bias and
                    # folding the 1/sqrt(HD) score scale into q rows (DVE)
                    if j < 4:
                        nc.vector.tensor_scalar(
                            qk_sb[j][:], ps[:], wbp_sb[:, j : j + 1], 0.125,
                            op0=mybir.AluOpType.add, op1=mybir.AluOpType.mult,
                        )
                    else:
                        nc.vector.tensor_scalar_add(
                            qk_sb[j][:], ps[:], wbp_sb[:, j : j + 1]
                        )

                return mm, fin

            def v_blk(t):
                ps = ps_sm.tile([128, HPC * HD], F32, tag="sm", name=f"vps{t}")

                def mm(c):
                    nc.tensor.matmul(
                        ps[:],
                        hT_sb[c][:, t * 128 : (t + 1) * 128],
                        wT_sb[c][:, 2 * HPC * HD : 3 * HPC * HD],
                        start=(c == 0),
                        stop=(c == DC - 1),
                    )

                def fin():
                    nc.vector.tensor_tensor(
                        v_sb[t][:, :, 0:HD],
                        ps[:].rearrange("p (h d) -> p h d", h=HPC),
                        wbv_sb[:],
                        op=mybir.AluOpType.add,
                    )
                    nc.scalar.activation(
                        v_sb[t][:, :, HD : HD + 1],
                        v_sb[t][:, :, 0:1],
                        mybir.ActivationFunctionType.Identity,
                        scale=0.0,
                        bias=1.0,
                    )

                return mm, fin

            bands = [
                [qk_blk(0), qk_blk(4), v_blk(0), v_blk(1), v_blk(2)],
                [v_blk(3), v_blk(4), v_blk(5), qk_blk(1), qk_blk(5)],
                [v_blk(6), v_blk(7), qk_blk(2), qk_blk(6)],
                [qk_blk(3), qk_blk(7)],
            ]
            for band in bands:
                for c in range(DC):
                    for mm, _ in band:
                        mm(c)
                for _, fin in band:
                    fin()

            # --- phase 2: attention ----------------------------------------
            # Software-pipelined across (slot, k-chunk) items: the AV matmuls
            # are emitted LAG items late so the in-order PE stream never
            # stalls waiting on an item's exp * exp(bias).
            #
            # Dense slots (r=7) accumulate kc-major with full-half groups.
            # Sparse slots accumulate qc-major: per query-block, one
            # uniform-footprint PSUM group over its alive key-chunks
            # (region-varying start/stop within a group loses contributions).
            # Slots are processed in interleaved PAIRS (a,b): [a.kc0, b.kc0,
            # a.kc1, ...]. Pairing a ScalarE-heavy dense slot with a PE-cheap
            # sparse slot keeps both engines below the pipeline rate, and a
            # pair's two pos tiles exactly fill the 4-buffer PSUM rotation.
            order = sorted(range(HPC), key=lambda s: (-slot_r[s], s))
            pairs2 = [(order[i], order[HPC - 1 - i]) for i in range(HPC // 2)]
            items = []
            for a, b in pairs2:
                for kc in range(KC):
                    items.append((a, kc))
                    items.append((b, kc))
            idx_of = {it: i for i, it in enumerate(items)}
            et2s: dict[tuple, object] = {}
            pos_map: dict[int, list] = {}

            def emit_front(i):
                s, kc = items[i]
                r = slot_r[s]
                lo, hi = _window(r, kc)
                w0, w1 = lo * 128, (hi + 1) * 128
                j, po = s // 2, (s % 2) * 64
                qT = qk_sb[j][po : po + 64, :]  # [64, S] (already /8)
                kT = qk_sb[4 + j][po : po + 64, :]  # [64, S]
                ebt = ebp.tile([128, S], BF16, tag="eb", name=f"eb{i}")
                nc.sync.dma_start(
                    out=ebt[:, 0 : w1 - w0],
                    in_=ebT[s, kc * 128 : (kc + 1) * 128, w0:w1],
                )
                ps = ps_mm.tile([128, S], F32, tag="mm", name=f"s{i}")
                # scoresT[k, q] = k @ q.T over the alive window only
                for _hb, c0, c1 in _half_ranges(w0, w1):
                    nc.tensor.matmul(
                        ps[:, c0:c1],
                        kT[:, kc * 128 : (kc + 1) * 128],
                        qT[:, c0:c1],
                        start=True,
                        stop=True,
                    )
                et = ep.tile([128, S], BF16, tag="et", name=f"et{i}")
                nc.scalar.activation(
                    et[:, w0:w1], ps[:, w0:w1], mybir.ActivationFunctionType.Exp
                )
                # fold in the additive bias: exp(s+b) = exp(s)*exp(b)  (DVE)
                et2 = e2p.tile([128, S], BF16, tag="e2", name=f"e2_{i}")
                nc.vector.tensor_tensor(
                    et2[:, w0:w1],
                    et[:, w0:w1],
                    ebt[:, 0 : w1 - w0],
                    op=mybir.AluOpType.mult,
                )
                et2s[(s, kc)] = et2

            def get_pos(s):
                if s not in pos_map:
                    # [65, 512] 1-bank output tiles: rows 0..63 = outT,
                    # row 64 = sum of exp over the alive band
                    pos_map[s] = [
                        ps_sm.tile([HD + 1, 512], F32, tag="sm", name=f"po{s}_{k}")
                        for k in range(2)
                    ]
                return pos_map[s]

            def emit_back_dense(s, kc):
                pos = get_pos(s)
                et2 = et2s[(s, kc)]
                for hb in range(2):
                    nc.tensor.matmul(
                        pos[hb][:],
                        v_sb[kc][:, s, :],
                        et2[:, hb * 512 : (hb + 1) * 512],
                        start=(kc == 0),
                        stop=(kc == KC - 1),
                    )
                if kc == KC - 1:
                    for kc2 in range(KC):
                        del et2s[(s, kc2)]
                    emit_tail(s, 0)
                    emit_tail(s, 1)

            def emit_back_sparse(s, qc):
                r = slot_r[s]
                pos = get_pos(s)
                hb, c0 = qc // 4, (qc % 4) * 128
                klo, khi = max(0, qc - r), min(KC - 1, qc + r)
                for kc in range(klo, khi + 1):
                    nc.tensor.matmul(
                        pos[hb][:, c0 : c0 + 128],
                        v_sb[kc][:, s, :],
                        et2s[(s, kc)][:, qc * 128 : (qc + 1) * 128],
                        start=(kc == klo),
                        stop=(kc == khi),
                    )
                if qc == 3:
                    # queries 0..511 complete: normalize half 0 early
                    emit_tail(s, 0)
                if qc == KC - 1:
                    for kc in range(KC):
                        del et2s[(s, kc)]
                    emit_tail(s, 1)

            def emit_tail(s, half):
                # normalize: out[d,q] * (1/sum[q]).  1/sum via
                # reciprocal_approx_fast (18 bits; sums are benign), broadcast
                # along partitions on the idle GpSimd, multiply on DVE.
                if True:
                    p = get_pos(s)[half]
                    # the sum row lives at PSUM partition 64; DVE cannot
                    # read partition 64 into partition 0, ScalarE can
                    smf = op_.tile([1, 512], F32, tag="smf")
                    nc.scalar.activation(
                        smf[:], p[HD : HD + 1, :],
                        mybir.ActivationFunctionType.Copy,
                    )
                    rcf = op_.tile([1, 512], F32, tag="rcf")
                    nc.vector.reciprocal_approx_fast(rcf[:], smf[:])
                    rb = op_.tile([HD, 512], F32, tag="rb")
                    nc.gpsimd.partition_broadcast(rb[:], rcf[:])
                    ot = op_.tile([HD, 512], F32, tag="ot")
                    nc.vector.tensor_tensor(
                        ot[:], p[0:HD, :], rb[:], op=mybir.AluOpType.mult
                    )
                    nc.sync.dma_start(
                        out=oT[
                            s * HD : (s + 1) * HD, half * 512 : (half + 1) * 512
                        ],
                        in_=ot[:],
                    )

            # backs: (emit-at front index, emit fn). Dense back (s,kc) needs
            # front (s,kc); sparse back (s,qc) needs fronts through qc+r.
            # Sparse items are small (short exp/mult chains), so they ride
            # closer behind their fronts. Normalize tails are deferred a
            # couple of items past their last AV so the ScalarE copy / DVE
            # reciprocal never block the in-order engine queues waiting for
            # the AV accumulation to finish.
            backs = []
            last_pair = set(pairs2[-1])
            for s in range(HPC):
                r = slot_r[s]
                # deep lag overlaps a pair's drain with the next pair's
                # fronts; the final pair has nothing to overlap with, so it
                # rides close behind its fronts instead
                dl, sl = (4, 3) if s in last_pair else (10, 9)
                if r >= KC - 1:
                    for kc in range(KC):
                        backs.append(
                            (
                                idx_of[(s, kc)] + dl,
                                lambda s=s, kc=kc: emit_back_dense(s, kc),
                            )
                        )
                else:
                    for qc in range(KC):
                        backs.append(
                            (
                                idx_of[(s, min(KC - 1, qc + r))] + sl,
                                lambda s=s, qc=qc: emit_back_sparse(s, qc),
                            )
                        )
            backs.sort(key=lambda b: b[0])
            bi = 0
            for i in range(len(items)):
                emit_front(i)
                while bi < len(backs) and backs[bi][0] <= i:
                    backs[bi][1]()
                    bi += 1
            while bi < len(backs):
                backs[bi][1]()
                bi += 1

    # Bacc defers register allocation to its compile() pass, which only runs
    # in finalize(); run_bass_via_pjrt ships the BIR as-is, so finalize here.
    nc.finalize()
    return nc


def shard_inputs(hidden_states, bias, Wqkv_w, Wqkv_b):
    """Slice + lay out the full inputs into 8 per-core input maps."""
    import ml_dtypes

    bf16 = ml_dtypes.bfloat16
    hidden_states = np.asarray(hidden_states, dtype=np.float32)
    bias = np.asarray(bias, dtype=np.float32)
    Wqkv_w = np.asarray(Wqkv_w, dtype=np.float32)
    Wqkv_b = np.asarray(Wqkv_b, dtype=np.float32)

    in_maps = []
    for c in range(N_CORES):
        b, par = c // 2, c % 2
        heads = [PAIRS[s][par] for s in range(HPC)]
        rows = np.concatenate(
            [np.arange(g * D + h * HD, g * D + (h + 1) * HD) for g in range(3) for h in heads]
        )
        wbp2 = np.ascontiguousarray(
            Wqkv_b[rows[0 : 2 * HPC * HD]].reshape(8, 128).T
        ).astype(np.float32)
        wbv2 = np.broadcast_to(
            Wqkv_b[rows[2 * HPC * HD :]].astype(bf16)[None, :], (128, HPC * HD)
        )
        eb = np.exp(bias[b, heads])  # [8, S, S] fp32
        in_maps.append(
            {
                "hw": np.concatenate(
                    [hidden_states[b].T, Wqkv_w[rows].T], axis=1
                ).astype(bf16),
                "wbv": np.ascontiguousarray(wbv2),
                "wbp": wbp2,
                "ebT": np.ascontiguousarray(eb.transpose(0, 2, 1)).astype(bf16),
            }
        )
    return in_maps


def gather(res):
    out = np.empty((B, S, D), dtype=np.float32)
    for c in range(N_CORES):
        b, par = c // 2, c % 2
        for s in range(HPC):
            h = PAIRS[s][par]
            out[b, :, h * HD : (h + 1) * HD] = res.results[c]["oT"][
                s * HD : (s + 1) * HD, :
            ].T
    return out


def sparsity_ok(bias) -> bool:
    """Verify the ACTUAL bias values keep every skipped block below -T_CHECK
    (so its softmax mass is < ~e^-12 of the row total)."""
    if all(r >= KC - 1 for r in SLOT_R):
        return True
    bias = np.asarray(bias, dtype=np.float32)
    bm = bias.reshape(B, H, KC, 128, KC, 128).max(axis=(3, 5))  # [B,H,8,8]
    kc = np.arange(KC)[:, None]
    qc = np.arange(KC)[None, :]
    for s, r in enumerate(SLOT_R):
        dead = np.abs(kc - qc) > r
        if not dead.any():
            continue
        for h in PAIRS[s]:
            if not np.all(bm[:, h][:, dead] < -T_CHECK):
                return False
    return True


_CACHED = {}


def kernel(hidden_states, bias, Wqkv_w, Wqkv_b):
    from concourse.bass_utils import run_bass_kernel_spmd

    slot_r = tuple(SLOT_R if sparsity_ok(bias) else DENSE_R)
    if slot_r not in _CACHED:
        _CACHED[slot_r] = build_bass(list(slot_r))
    in_maps = shard_inputs(hidden_states, bias, Wqkv_w, Wqkv_b)
    res = run_bass_kernel_spmd(
        _CACHED[slot_r], in_maps, core_ids=list(range(N_CORES))
    )
    return gather(res)
